# revision 24
# baseline (speedup 1.0000x reference)
"""LocalitySelfAttention TRN2 kernel (v6: flat cross-block pipeline).

B=4, N=2048, C=768, H=12, D=64.  8 cores: core c -> batch c//2, heads
6*(c%2) .. 6*(c%2)+6 (6 contiguous heads = 3 pairs).  Each core computes its
heads' qkv projection, attention, and a partial output projection restricted
to its heads' 384 rows of w_proj.  Host sums the two partials per batch and
adds b_proj.

The whole phase-2 is ONE flat software pipeline over (pair, q-half, kb)
iterations with the AV matmuls lagging the score matmuls by exactly one
iteration, including across block boundaries.  This keeps the PE's strict
in-order queue free of semaphore waits (an AV never reaches the queue head
before its exp finished) and keeps PE activity dense enough that the HAM
clock gate stays at full rate.  Softmax normalization runs entirely off the
critical path: denominator rows round-trip through DRAM (reshaped [8,128]
so the DVE reciprocal runs on 8 partitions at free=128), with the dependent
DVE ops deferred half an iteration-block so their DMA waits are always
pre-satisfied when they reach the strict-FIFO DVE queue.
"""

import sys
import numpy as np

if "/opt/trn_rl_repo" not in sys.path:
    sys.path.insert(0, "/opt/trn_rl_repo")

B, N, C, H = 4, 2048, 768, 12
D = C // H          # 64
NH = 6              # heads per core
NP = NH // 2        # head pairs per core = 3
P = 128
CT = C // P         # 6 contraction tiles
KB = N // P         # 16 key blocks
QC = N // 512       # 4 free-dim chunks of 512
HF = 1024           # q processed in halves
SCALE = float(D) ** -0.5  # 0.125

_CACHE = {}


def _build_program():
    import concourse.bass as bass
    import concourse.mybir as mybir
    import concourse.tile as tile
    from concourse import bacc
    from concourse.masks import make_identity

    f32 = mybir.dt.float32
    bf16 = mybir.dt.bfloat16
    Exp = mybir.ActivationFunctionType.Exp
    mult = mybir.AluOpType.mult
    add = mybir.AluOpType.add

    nc = bacc.Bacc()
    xT = nc.dram_tensor("xT", [C, N], bf16, kind="ExternalInput")
    wqkv = nc.dram_tensor("wqkv", [C, 3 * NH * D], bf16, kind="ExternalInput")
    wproj = nc.dram_tensor("wproj", [NH * D, C], bf16, kind="ExternalInput")
    temp = nc.dram_tensor("temp", [P, NH], f32, kind="ExternalInput")
    outT = nc.dram_tensor("outT", [C, N], f32, kind="ExternalOutput")
    rdram_s = nc.dram_tensor("rscratch_s", [2 * NH, HF], f32)  # denom rows
    rdram_r = nc.dram_tensor("rscratch_r", [2 * NH, HF], f32)  # recip rows

    mm = nc.tensor.matmul

    with tile.TileContext(nc) as tc:
        with (
            tc.tile_pool(name="const", bufs=1) as constp,
            tc.tile_pool(name="persist", bufs=1) as persist,
        ):
            # ---- setup: temperature diag masks (1 - t_h * I) ----------
            ident = constp.tile([P, P], f32, tag="ident")
            make_identity(nc, ident[:])
            tbc = constp.tile([P, NH], f32, tag="tbc")
            nc.sync.dma_start(tbc[:, :], temp[:, :])
            ntb = constp.tile([P, NH], f32, tag="ntb")
            nc.vector.tensor_scalar_mul(ntb[:, :], tbc[:, :], -1.0)
            masks = constp.tile([P, NH, P], f32, tag="masks")
            for h in range(NH):
                nc.vector.tensor_scalar(
                    masks[:, h, :], ident[:], ntb[:, h : h + 1], 1.0, mult, add
                )

            # persistent: qT/kT (head pairs stacked on partitions), v_aug
            qkT = persist.tile([P, 2 * NP, N], bf16, tag="qkT")  # 0-2 q, 3-5 k
            vaug = persist.tile([P, KB, NH, D + 1], bf16, tag="vaug")
            onesrc = constp.tile([P, KB * NH], f32, tag="onesrc")
            nc.vector.memset(onesrc[:], 1.0)
            nc.vector.tensor_copy(
                vaug[:, :, :, D : D + 1],
                onesrc[:].rearrange("p (a b c) -> p a b c", a=KB, b=NH),
            )
            wp = persist.tile([P, NH * D // P, C], bf16, tag="wp")  # [128,3,768]
            for g3 in range(NH * D // P):
                nc.sync.dma_start(wp[:, g3, :], wproj[g3 * P : (g3 + 1) * P, :])
            attnT = persist.tile([P, NP, N], bf16, tag="attnT")

            # ---- phase 1: qkv projection ------------------------------
            with (
                tc.tile_pool(name="qin", bufs=1) as qin,
                tc.tile_pool(name="psum1", bufs=2, space=bass.MemorySpace.PSUM) as psum1,
            ):
                xts, wqs = [], []
                for t in range(CT):
                    xti = qin.tile([P, N], bf16, tag=f"xt{t}")
                    nc.sync.dma_start(xti[:], xT[t * P : (t + 1) * P, :])
                    xts.append(xti)
                    wqi = qin.tile([P, 3 * NH * D], bf16, tag=f"wq{t}")
                    nc.sync.dma_start(wqi[:], wqkv[t * P : (t + 1) * P, :])
                    wqs.append(wqi)

                def qk_group(g):
                    ps = psum1.tile([P, N], f32, tag="ps", name=f"ps{g}")
                    for t in range(CT):
                        for qc in range(QC):
                            mm(
                                ps[:, qc * 512 : (qc + 1) * 512],
                                wqs[t][:, g * P : (g + 1) * P],
                                xts[t][:, qc * 512 : (qc + 1) * 512],
                                start=(t == 0),
                                stop=(t == CT - 1),
                            )
                    nc.vector.tensor_copy(qkT[:, g, :], ps[:])

                def v_group(rb_i):
                    psv = psum1.tile([P, NH * D], f32, tag="ps", name=f"psv{rb_i}")
                    for t in range(CT):
                        mm(
                            psv[:],
                            xts[t][:, rb_i * P : (rb_i + 1) * P],
                            wqs[t][:, 2 * NH * D : 3 * NH * D],
                            start=(t == 0),
                            stop=(t == CT - 1),
                        )
                    nc.vector.tensor_copy(
                        vaug[:, rb_i, :, 0:D],
                        psv[:].rearrange("p (h d) -> p h d", h=NH),
                    )

                # pair-0's q (g=0) and k (g=3) first so phase 2 starts
                # early; v groups interleaved so their DVE copies hide
                # under the big qk matmul groups.
                qk_group(0)
                for i in range(4):
                    v_group(i)
                qk_group(3)
                for i in range(4, 8):
                    v_group(i)
                qk_group(1)
                for i in range(8, 12):
                    v_group(i)
                qk_group(4)
                for i in range(12, 16):
                    v_group(i)
                qk_group(2)
                qk_group(5)

            # ---- phase 2: attention, one flat pipeline ----------------
            with (
                tc.tile_pool(name="pt", bufs=4) as ptp,
                tc.tile_pool(name="un", bufs=4) as unp,
                tc.tile_pool(name="rb", bufs=2) as rbp,
                tc.tile_pool(name="pst", bufs=2, space=bass.MemorySpace.PSUM) as pst,
                tc.tile_pool(name="pav", bufs=2, space=bass.MemorySpace.PSUM) as pav,
            ):
                blocks = [(p, hf) for p in range(NP) for hf in range(2)]
                avs = [None] * len(blocks)
                prev = None      # (bi, kb, ptA, ptB)
                pending = []     # deferred normalize tails

                def emit_av(bi, kb, ptA, ptB):
                    p, hf = blocks[bi]
                    avA, avB = avs[bi]
                    for qc in range(2):
                        cs = slice(qc * 512, (qc + 1) * 512)
                        mm(avA[:, cs], vaug[:, kb, 2 * p, :], ptA[:, cs],
                           start=(kb == 0), stop=(kb == KB - 1))
                        mm(avB[:, cs], vaug[:, kb, 2 * p + 1, :], ptB[:, cs],
                           start=(kb == 0), stop=(kb == KB - 1))
                    if kb == KB - 1:
                        emit_norm_a(bi)

                def emit_norm_a(bi):
                    # copy av out of PSUM (frees the accumulator slot), ship
                    # the denominator row to DRAM; the rest is deferred.
                    p, hf = blocks[bi]
                    q0 = hf * HF
                    avA, avB = avs[bi]
                    for off, avX, h in ((0, avA, 2 * p), (D, avB, 2 * p + 1)):
                        un = unp.tile([D + 1, HF], f32, tag="un",
                                      name=f"un{bi}_{h}")
                        nc.vector.tensor_copy(un[:], avX[:])
                        ri = 2 * h + hf
                        nc.sync.dma_start(rdram_s[ri, :], un[D : D + 1, :])
                        pending.append(
                            lambda un=un, ri=ri, off=off, p=p, q0=q0:
                            emit_norm_b(un, ri, off, p, q0)
                        )

                def emit_norm_b(un, ri, off, p, q0):
                    rp = rbp.tile([8, P], f32, tag="rp", name=f"rp{ri}")
                    nc.sync.dma_start(
                        rp[0:8, :],
                        rdram_s[ri, :].rearrange("(a b) -> a b", a=8),
                    )
                    nc.vector.reciprocal(rp[0:8, :], rp[0:8, :])
                    nc.sync.dma_start(rdram_r[ri, :], rp[0:8, :])
                    rb = rbp.tile([D, HF], f32, tag="rb", name=f"rb{ri}")
                    nc.sync.dma_start(
                        rb[:],
                        rdram_r[ri : ri + 1, :].broadcast_to([D, HF]),
                    )
                    nc.vector.tensor_mul(
                        attnT[off : off + D, p, q0 : q0 + HF],
                        un[0:D, :],
                        rb[:],
                    )

                for bi, (p, hf) in enumerate(blocks):
                    q0 = hf * HF
                    hA, hB = 2 * p, 2 * p + 1
                    avs[bi] = (
                        pav.tile([D + 1, HF], f32, tag="av", name=f"avA{bi}"),
                        pav.tile([D + 1, HF], f32, tag="av", name=f"avB{bi}"),
                    )
                    for kb in range(KB):
                        if kb == 6 and pending:
                            for fn in pending:
                                fn()
                            pending = []
                        stA = pst.tile([P, HF], f32, tag="st", name=f"stA{bi}_{kb}")
                        stB = pst.tile([P, HF], f32, tag="st", name=f"stB{bi}_{kb}")
                        for qc in range(2):
                            cs = slice(qc * 512, (qc + 1) * 512)
                            qs = slice(q0 + qc * 512, q0 + (qc + 1) * 512)
                            ks = slice(kb * P, (kb + 1) * P)
                            mm(stA[:, cs], qkT[0:D, NP + p, ks],
                               qkT[0:D, p, qs], start=True, stop=True)
                            mm(stB[:, cs], qkT[D:P, NP + p, ks],
                               qkT[D:P, p, qs], start=True, stop=True)
                        if kb * P // HF == hf:
                            dcol = kb * P - q0
                            dsl = slice(dcol, dcol + P)
                            nc.vector.tensor_mul(
                                stA[:, dsl], stA[:, dsl], masks[:, hA, :]
                            )
                            nc.vector.tensor_mul(
                                stB[:, dsl], stB[:, dsl], masks[:, hB, :]
                            )
                        ptA = ptp.tile([P, HF], bf16, tag="pt", name=f"ptA{bi}_{kb}")
                        nc.scalar.activation(ptA[:], stA[:], Exp, scale=SCALE)
                        ptB = ptp.tile([P, HF], bf16, tag="pt", name=f"ptB{bi}_{kb}")
                        nc.scalar.activation(ptB[:], stB[:], Exp, scale=SCALE)
                        if prev is not None:
                            emit_av(*prev)
                        prev = (bi, kb, ptA, ptB)
                # drain: final AV flush + its normalize
                emit_av(*prev)
                for fn in pending:
                    fn()
                pending = []

            # ---- phase 3: output projection (transposed) --------------
            # interleave accumulation groups of m-tile pairs so the
            # dependency on the last pair's attnT (g3 == 2) stalls at
            # most once while other matmuls fill the queue.
            with (
                tc.tile_pool(name="psum3", bufs=2, space=bass.MemorySpace.PSUM) as psum3,
                tc.tile_pool(name="ot", bufs=2) as otp,
            ):
                for m0 in range(0, CT, 2):
                    pos = []
                    for m in (m0, m0 + 1):
                        po = psum3.tile([P, N], f32, tag="ps", name=f"po{m}")
                        pos.append(po)
                        for g3 in range(2):
                            for qc in range(QC):
                                cs = slice(qc * 512, (qc + 1) * 512)
                                mm(po[:, cs], wp[:, g3, m * P : (m + 1) * P],
                                   attnT[:, g3, cs],
                                   start=(g3 == 0), stop=False)
                    for i, m in enumerate((m0, m0 + 1)):
                        for qc in range(QC):
                            cs = slice(qc * 512, (qc + 1) * 512)
                            mm(pos[i][:, cs], wp[:, 2, m * P : (m + 1) * P],
                               attnT[:, 2, cs], start=False, stop=True)
                    for i, m in enumerate((m0, m0 + 1)):
                        ot = otp.tile([P, N], f32, tag="ot", name=f"ot{m}")
                        nc.vector.tensor_copy(ot[:], pos[i][:])
                        nc.sync.dma_start(outT[m * P : (m + 1) * P, :], ot[:])

    if not nc.is_finalized():
        nc.finalize()
    return nc


def _get_program():
    if "nc" not in _CACHE:
        _CACHE["nc"] = _build_program()
    return _CACHE["nc"]


def _in_maps(x, w_qkv, w_proj, temperature):
    import ml_dtypes

    bf16 = ml_dtypes.bfloat16
    t = np.asarray(temperature, dtype=np.float32).reshape(H)
    maps = []
    xTs = {}
    for c in range(8):
        b, h0 = c // 2, NH * (c % 2)
        if b not in xTs:
            xTs[b] = np.ascontiguousarray(
                np.asarray(x[b], dtype=np.float32).T.astype(bf16)
            )
        cols = slice(D * h0, D * h0 + NH * D)
        wq = np.concatenate(
            [w_qkv[:, cols], w_qkv[:, C:][:, cols], w_qkv[:, 2 * C :][:, cols]],
            axis=1,
        )
        maps.append(
            {
                "xT": xTs[b],
                "wqkv": np.ascontiguousarray(wq).astype(bf16),
                "wproj": np.ascontiguousarray(
                    w_proj[D * h0 : D * h0 + NH * D, :]
                ).astype(bf16),
                "temp": np.ascontiguousarray(
                    np.broadcast_to(t[h0 : h0 + NH].reshape(1, NH), (P, NH))
                ),
            }
        )
    return maps


def _install_profile_hook():
    """The agent image's antenv lacks axon_hooks; synthesize it and register
    the ctypes NTFF hook so run_bass_kernel_spmd(trace=True) can profile."""
    import types, importlib

    if "antenv.axon_hooks" not in sys.modules:
        import antenv

        mod = types.ModuleType("antenv.axon_hooks")
        _state = {"hook": None}
        mod.set_axon_ntff_profile_hook = lambda h: _state.__setitem__("hook", h)
        mod.get_axon_ntff_profile_hook = lambda: _state["hook"]
        sys.modules["antenv.axon_hooks"] = mod
        antenv.axon_hooks = mod
    from antenv.axon_hooks import (
        get_axon_ntff_profile_hook,
        set_axon_ntff_profile_hook,
    )

    if get_axon_ntff_profile_hook() is None:
        tb = importlib.import_module("trn_agent_boot.trn_boot")
        hook = tb._ntff_profile_via_ctypes("/opt/axon/libaxon_pjrt.so")
        set_axon_ntff_profile_hook(hook)


def kernel(x, w_qkv, w_proj, b_proj, temperature, _trace=False):
    from concourse.bass_utils import run_bass_kernel_spmd

    if _trace:
        try:
            _install_profile_hook()
        except Exception as e:  # profiling is best-effort
            print(f"profile hook install failed: {e}")

    nc = _get_program()
    maps = _in_maps(
        np.asarray(x, np.float32),
        np.asarray(w_qkv, np.float32),
        np.asarray(w_proj, np.float32),
        np.asarray(temperature, np.float32),
    )
    res = run_bass_kernel_spmd(nc, maps, list(range(8)), trace=_trace)
    parts = [r["outT"] for r in res.results]
    bp = np.asarray(b_proj, np.float32)
    out = np.stack(
        [(parts[2 * b] + parts[2 * b + 1]).T + bp for b in range(B)]
    ).astype(np.float32)
    if _trace:
        _CACHE["last_result"] = res
    return out


# revision 28
# speedup vs baseline: 1.2591x; 1.2591x over previous
"""LocalitySelfAttention TRN2 kernel (v6: flat cross-block pipeline).

B=4, N=2048, C=768, H=12, D=64.  8 cores: core c -> batch c//2, heads
6*(c%2) .. 6*(c%2)+6 (6 contiguous heads = 3 pairs).  Each core computes its
heads' qkv projection, attention, and a partial output projection restricted
to its heads' 384 rows of w_proj.  Host sums the two partials per batch and
adds b_proj.

The whole phase-2 is ONE flat software pipeline over (pair, q-half, kb)
iterations with the AV matmuls lagging the score matmuls by exactly one
iteration, including across block boundaries.  This keeps the PE's strict
in-order queue free of semaphore waits (an AV never reaches the queue head
before its exp finished) and keeps PE activity dense enough that the HAM
clock gate stays at full rate.  Softmax normalization runs entirely off the
critical path: denominator rows round-trip through DRAM (reshaped [8,128]
so the DVE reciprocal runs on 8 partitions at free=128), with the dependent
DVE ops deferred half an iteration-block so their DMA waits are always
pre-satisfied when they reach the strict-FIFO DVE queue.
"""

import sys
import numpy as np

if "/opt/trn_rl_repo" not in sys.path:
    sys.path.insert(0, "/opt/trn_rl_repo")

B, N, C, H = 4, 2048, 768, 12
D = C // H          # 64
NH = 6              # heads per core
NP = NH // 2        # head pairs per core = 3
P = 128
CT = C // P         # 6 contraction tiles
KB = N // P         # 16 key blocks
QC = N // 512       # 4 free-dim chunks of 512
HF = 1024           # q processed in halves
SCALE = float(D) ** -0.5  # 0.125

_CACHE = {}


def _build_program():
    import concourse.bass as bass
    import concourse.mybir as mybir
    import concourse.tile as tile
    from concourse import bacc
    from concourse.masks import make_identity

    f32 = mybir.dt.float32
    bf16 = mybir.dt.bfloat16
    Exp = mybir.ActivationFunctionType.Exp
    mult = mybir.AluOpType.mult
    add = mybir.AluOpType.add

    nc = bacc.Bacc()
    xT = nc.dram_tensor("xT", [C, N], bf16, kind="ExternalInput")
    wqkv = nc.dram_tensor("wqkv", [C, 3 * NH * D], bf16, kind="ExternalInput")
    wproj = nc.dram_tensor("wproj", [NH * D, C], bf16, kind="ExternalInput")
    temp = nc.dram_tensor("temp", [P, NH], f32, kind="ExternalInput")
    outT = nc.dram_tensor("outT", [C, N], f32, kind="ExternalOutput")
    rdram_s = nc.dram_tensor("rscratch_s", [2 * NH, HF], f32)  # denom rows
    rdram_r = nc.dram_tensor("rscratch_r", [2 * NH, HF], f32)  # recip rows

    mm = nc.tensor.matmul

    with tile.TileContext(nc) as tc:
        with (
            tc.tile_pool(name="const", bufs=1) as constp,
            tc.tile_pool(name="persist", bufs=1) as persist,
        ):
            # ---- setup: temperature diag masks (1 - t_h * I) ----------
            ident = constp.tile([P, P], f32, tag="ident")
            make_identity(nc, ident[:])
            tbc = constp.tile([P, NH], f32, tag="tbc")
            nc.sync.dma_start(tbc[:, :], temp[:, :])
            ntb = constp.tile([P, NH], f32, tag="ntb")
            nc.vector.tensor_scalar_mul(ntb[:, :], tbc[:, :], -1.0)
            masks = constp.tile([P, NH, P], f32, tag="masks")
            for h in range(NH):
                nc.vector.tensor_scalar(
                    masks[:, h, :], ident[:], ntb[:, h : h + 1], 1.0, mult, add
                )

            # persistent: qT/kT (head pairs stacked on partitions), v_aug
            qkT = persist.tile([P, 2 * NP, N], bf16, tag="qkT")  # 0-2 q, 3-5 k
            vaug = persist.tile([P, KB, NH, D + 1], bf16, tag="vaug")
            onesrc = constp.tile([P, KB * NH], f32, tag="onesrc")
            nc.vector.memset(onesrc[:], 1.0)
            nc.vector.tensor_copy(
                vaug[:, :, :, D : D + 1],
                onesrc[:].rearrange("p (a b c) -> p a b c", a=KB, b=NH),
            )
            wp = persist.tile([P, NH * D // P, C], bf16, tag="wp")  # [128,3,768]
            for g3 in range(NH * D // P):
                nc.sync.dma_start(wp[:, g3, :], wproj[g3 * P : (g3 + 1) * P, :])
            attnT = persist.tile([P, NP, N], bf16, tag="attnT")

            # ---- phase 1 (prefix): v + pair-0's q/k only --------------
            # The remaining q/k projection groups are injected into the
            # phase-2 pipeline at intervals: each injection is a dense,
            # exp-independent PE burst that re-warms the HAM clock gate
            # (an ACT-bound steady state alone never has a 3.4us
            # contiguous-busy window, so the PE would stay cold forever).
            qin_cm = tc.tile_pool(name="qin", bufs=1)
            qin = qin_cm.__enter__()
            xts, wqs = [], []
            for t in range(CT):
                xti = qin.tile([P, N], bf16, tag=f"xt{t}", name=f"xt{t}")
                nc.sync.dma_start(xti[:], xT[t * P : (t + 1) * P, :])
                xts.append(xti)
                wqi = qin.tile([P, 3 * NH * D], bf16, tag=f"wq{t}", name=f"wq{t}")
                nc.sync.dma_start(wqi[:], wqkv[t * P : (t + 1) * P, :])
                wqs.append(wqi)

            with tc.tile_pool(name="psum1", bufs=2, space=bass.MemorySpace.PSUM) as psum1:

                def qk_group(g):
                    ps = psum1.tile([P, N], f32, tag="ps", name=f"ps{g}")
                    for t in range(CT):
                        for qc in range(QC):
                            mm(
                                ps[:, qc * 512 : (qc + 1) * 512],
                                wqs[t][:, g * P : (g + 1) * P],
                                xts[t][:, qc * 512 : (qc + 1) * 512],
                                start=(t == 0),
                                stop=(t == CT - 1),
                            )
                    nc.vector.tensor_copy(qkT[:, g, :], ps[:])

                def v_group(rb_i):
                    psv = psum1.tile([P, NH * D], f32, tag="ps", name=f"psv{rb_i}")
                    for t in range(CT):
                        mm(
                            psv[:],
                            xts[t][:, rb_i * P : (rb_i + 1) * P],
                            wqs[t][:, 2 * NH * D : 3 * NH * D],
                            start=(t == 0),
                            stop=(t == CT - 1),
                        )
                    nc.vector.tensor_copy(
                        vaug[:, rb_i, :, 0:D],
                        psv[:].rearrange("p (h d) -> p h d", h=NH),
                    )

                qk_group(0)
                for i in range(8):
                    v_group(i)
                qk_group(3)
                for i in range(8, 16):
                    v_group(i)

            # ---- phase 2: attention, one flat pipeline ----------------
            with (
                tc.tile_pool(name="pt", bufs=4) as ptp,
                tc.tile_pool(name="un", bufs=4) as unp,
                tc.tile_pool(name="rb", bufs=2) as rbp,
                tc.tile_pool(name="pst", bufs=2, space=bass.MemorySpace.PSUM) as pst,
                tc.tile_pool(name="pav", bufs=2, space=bass.MemorySpace.PSUM) as pav,
            ):
                def inject_qk(g, half):
                    # one [128, 1024] chunk of a q/k projection group,
                    # accumulated in an st-ring slot then copied to qkT
                    stq = pst.tile([P, HF], f32, tag="st", name=f"stq{g}_{half}")
                    for t in range(CT):
                        for qc in range(2):
                            cs = slice(qc * 512, (qc + 1) * 512)
                            xs = slice(half * HF + qc * 512,
                                       half * HF + (qc + 1) * 512)
                            mm(stq[:, cs], wqs[t][:, g * P : (g + 1) * P],
                               xts[t][:, xs], start=(t == 0), stop=(t == CT - 1))
                    nc.vector.tensor_copy(qkT[:, g, half * HF : (half + 1) * HF],
                                          stq[:])

                injections = {
                    (0, 5): (1, 0), (0, 11): (1, 1),
                    (1, 3): (4, 0), (1, 9): (4, 1),
                    (2, 5): (2, 0), (2, 11): (2, 1),
                    (3, 3): (5, 0), (3, 9): (5, 1),
                }
                blocks = [(p, hf) for p in range(NP) for hf in range(2)]
                avs = [None] * len(blocks)
                prev = None      # (bi, kb, ptA, ptB)
                pending = []     # deferred normalize tails

                def emit_av(bi, kb, ptA, ptB):
                    p, hf = blocks[bi]
                    avA, avB = avs[bi]
                    for qc in range(2):
                        cs = slice(qc * 512, (qc + 1) * 512)
                        mm(avA[:, cs], vaug[:, kb, 2 * p, :], ptA[:, cs],
                           start=(kb == 0), stop=(kb == KB - 1))
                        mm(avB[:, cs], vaug[:, kb, 2 * p + 1, :], ptB[:, cs],
                           start=(kb == 0), stop=(kb == KB - 1))
                    if kb == KB - 1:
                        emit_norm_a(bi)

                def emit_norm_a(bi):
                    # copy av out of PSUM (frees the accumulator slot), ship
                    # the denominator row to DRAM; the rest is deferred.
                    p, hf = blocks[bi]
                    q0 = hf * HF
                    avA, avB = avs[bi]
                    for off, avX, h in ((0, avA, 2 * p), (D, avB, 2 * p + 1)):
                        un = unp.tile([D + 1, HF], f32, tag="un",
                                      name=f"un{bi}_{h}")
                        nc.vector.tensor_copy(un[:], avX[:])
                        ri = 2 * h + hf
                        nc.sync.dma_start(rdram_s[ri, :], un[D : D + 1, :])
                        pending.append(
                            lambda un=un, ri=ri, off=off, p=p, q0=q0:
                            emit_norm_b(un, ri, off, p, q0)
                        )

                def emit_norm_b(un, ri, off, p, q0):
                    rp = rbp.tile([8, P], f32, tag="rp", name=f"rp{ri}")
                    nc.sync.dma_start(
                        rp[0:8, :],
                        rdram_s[ri, :].rearrange("(a b) -> a b", a=8),
                    )
                    nc.vector.reciprocal(rp[0:8, :], rp[0:8, :])
                    nc.sync.dma_start(rdram_r[ri, :], rp[0:8, :])
                    rb = rbp.tile([D, HF], f32, tag="rb", name=f"rb{ri}")
                    nc.sync.dma_start(
                        rb[:],
                        rdram_r[ri : ri + 1, :].broadcast_to([D, HF]),
                    )
                    nc.vector.tensor_mul(
                        attnT[off : off + D, p, q0 : q0 + HF],
                        un[0:D, :],
                        rb[:],
                    )

                for bi, (p, hf) in enumerate(blocks):
                    q0 = hf * HF
                    hA, hB = 2 * p, 2 * p + 1
                    avs[bi] = (
                        pav.tile([D + 1, HF], f32, tag="av", name=f"avA{bi}"),
                        pav.tile([D + 1, HF], f32, tag="av", name=f"avB{bi}"),
                    )
                    for kb in range(KB):
                        if (bi, kb) in injections:
                            inject_qk(*injections[(bi, kb)])
                        if kb in (12, 13) and pending:
                            fn = pending.pop(0)
                            fn()
                        stA = pst.tile([P, HF], f32, tag="st", name=f"stA{bi}_{kb}")
                        stB = pst.tile([P, HF], f32, tag="st", name=f"stB{bi}_{kb}")
                        for qc in range(2):
                            cs = slice(qc * 512, (qc + 1) * 512)
                            qs = slice(q0 + qc * 512, q0 + (qc + 1) * 512)
                            ks = slice(kb * P, (kb + 1) * P)
                            mm(stA[:, cs], qkT[0:D, NP + p, ks],
                               qkT[0:D, p, qs], start=True, stop=True)
                            mm(stB[:, cs], qkT[D:P, NP + p, ks],
                               qkT[D:P, p, qs], start=True, stop=True)
                        if kb * P // HF == hf:
                            dcol = kb * P - q0
                            dsl = slice(dcol, dcol + P)
                            nc.vector.tensor_mul(
                                stA[:, dsl], stA[:, dsl], masks[:, hA, :]
                            )
                            nc.vector.tensor_mul(
                                stB[:, dsl], stB[:, dsl], masks[:, hB, :]
                            )
                        ptA = ptp.tile([P, HF], bf16, tag="pt", name=f"ptA{bi}_{kb}")
                        nc.scalar.activation(ptA[:], stA[:], Exp, scale=SCALE)
                        ptB = ptp.tile([P, HF], bf16, tag="pt", name=f"ptB{bi}_{kb}")
                        nc.scalar.activation(ptB[:], stB[:], Exp, scale=SCALE)
                        if prev is not None:
                            emit_av(*prev)
                        prev = (bi, kb, ptA, ptB)
                # drain: final AV flush + its normalize
                emit_av(*prev)
                for fn in pending:
                    fn()
                pending = []
            qin_cm.__exit__(None, None, None)

            # ---- phase 3: output projection (transposed) --------------
            # interleave accumulation groups of m-tile pairs so the
            # dependency on the last pair's attnT (g3 == 2) stalls at
            # most once while other matmuls fill the queue.
            with (
                tc.tile_pool(name="psum3", bufs=2, space=bass.MemorySpace.PSUM) as psum3,
                tc.tile_pool(name="ot", bufs=2) as otp,
            ):
                for m0 in range(0, CT, 2):
                    pos = []
                    for m in (m0, m0 + 1):
                        po = psum3.tile([P, N], f32, tag="ps", name=f"po{m}")
                        pos.append(po)
                        for g3 in range(2):
                            for qc in range(QC):
                                cs = slice(qc * 512, (qc + 1) * 512)
                                mm(po[:, cs], wp[:, g3, m * P : (m + 1) * P],
                                   attnT[:, g3, cs],
                                   start=(g3 == 0), stop=False)
                    for i, m in enumerate((m0, m0 + 1)):
                        for qc in range(QC):
                            cs = slice(qc * 512, (qc + 1) * 512)
                            mm(pos[i][:, cs], wp[:, 2, m * P : (m + 1) * P],
                               attnT[:, 2, cs], start=False, stop=True)
                    for i, m in enumerate((m0, m0 + 1)):
                        ot = otp.tile([P, N], f32, tag="ot", name=f"ot{m}")
                        nc.vector.tensor_copy(ot[:], pos[i][:])
                        nc.sync.dma_start(outT[m * P : (m + 1) * P, :], ot[:])

    if not nc.is_finalized():
        nc.finalize()
    return nc


def _get_program():
    if "nc" not in _CACHE:
        _CACHE["nc"] = _build_program()
    return _CACHE["nc"]


def _in_maps(x, w_qkv, w_proj, temperature):
    import ml_dtypes

    bf16 = ml_dtypes.bfloat16
    t = np.asarray(temperature, dtype=np.float32).reshape(H)
    maps = []
    xTs = {}
    for c in range(8):
        b, h0 = c // 2, NH * (c % 2)
        if b not in xTs:
            xTs[b] = np.ascontiguousarray(
                np.asarray(x[b], dtype=np.float32).T.astype(bf16)
            )
        cols = slice(D * h0, D * h0 + NH * D)
        wq = np.concatenate(
            [w_qkv[:, cols], w_qkv[:, C:][:, cols], w_qkv[:, 2 * C :][:, cols]],
            axis=1,
        )
        maps.append(
            {
                "xT": xTs[b],
                "wqkv": np.ascontiguousarray(wq).astype(bf16),
                "wproj": np.ascontiguousarray(
                    w_proj[D * h0 : D * h0 + NH * D, :]
                ).astype(bf16),
                "temp": np.ascontiguousarray(
                    np.broadcast_to(t[h0 : h0 + NH].reshape(1, NH), (P, NH))
                ),
            }
        )
    return maps


def _install_profile_hook():
    """The agent image's antenv lacks axon_hooks; synthesize it and register
    the ctypes NTFF hook so run_bass_kernel_spmd(trace=True) can profile."""
    import types, importlib

    if "antenv.axon_hooks" not in sys.modules:
        import antenv

        mod = types.ModuleType("antenv.axon_hooks")
        _state = {"hook": None}
        mod.set_axon_ntff_profile_hook = lambda h: _state.__setitem__("hook", h)
        mod.get_axon_ntff_profile_hook = lambda: _state["hook"]
        sys.modules["antenv.axon_hooks"] = mod
        antenv.axon_hooks = mod
    from antenv.axon_hooks import (
        get_axon_ntff_profile_hook,
        set_axon_ntff_profile_hook,
    )

    if get_axon_ntff_profile_hook() is None:
        tb = importlib.import_module("trn_agent_boot.trn_boot")
        hook = tb._ntff_profile_via_ctypes("/opt/axon/libaxon_pjrt.so")
        set_axon_ntff_profile_hook(hook)


def kernel(x, w_qkv, w_proj, b_proj, temperature, _trace=False):
    from concourse.bass_utils import run_bass_kernel_spmd

    if _trace:
        try:
            _install_profile_hook()
        except Exception as e:  # profiling is best-effort
            print(f"profile hook install failed: {e}")

    nc = _get_program()
    maps = _in_maps(
        np.asarray(x, np.float32),
        np.asarray(w_qkv, np.float32),
        np.asarray(w_proj, np.float32),
        np.asarray(temperature, np.float32),
    )
    res = run_bass_kernel_spmd(nc, maps, list(range(8)), trace=_trace)
    parts = [r["outT"] for r in res.results]
    bp = np.asarray(b_proj, np.float32)
    out = np.stack(
        [(parts[2 * b] + parts[2 * b + 1]).T + bp for b in range(B)]
    ).astype(np.float32)
    if _trace:
        _CACHE["last_result"] = res
    return out


# revision 34
# speedup vs baseline: 1.3548x; 1.0760x over previous
"""LocalitySelfAttention TRN2 kernel (v6: flat cross-block pipeline).

B=4, N=2048, C=768, H=12, D=64.  8 cores: core c -> batch c//2, heads
6*(c%2) .. 6*(c%2)+6 (6 contiguous heads = 3 pairs).  Each core computes its
heads' qkv projection, attention, and a partial output projection restricted
to its heads' 384 rows of w_proj.  Host sums the two partials per batch and
adds b_proj.

The whole phase-2 is ONE flat software pipeline over (pair, q-half, kb)
iterations with the AV matmuls lagging the score matmuls by exactly one
iteration, including across block boundaries.  This keeps the PE's strict
in-order queue free of semaphore waits (an AV never reaches the queue head
before its exp finished) and keeps PE activity dense enough that the HAM
clock gate stays at full rate.  Softmax normalization runs entirely off the
critical path: denominator rows round-trip through DRAM (reshaped [8,128]
so the DVE reciprocal runs on 8 partitions at free=128), with the dependent
DVE ops deferred half an iteration-block so their DMA waits are always
pre-satisfied when they reach the strict-FIFO DVE queue.
"""

import sys
import numpy as np

if "/opt/trn_rl_repo" not in sys.path:
    sys.path.insert(0, "/opt/trn_rl_repo")

B, N, C, H = 4, 2048, 768, 12
D = C // H          # 64
NH = 6              # heads per core
NP = NH // 2        # head pairs per core = 3
P = 128
CT = C // P         # 6 contraction tiles
KB = N // P         # 16 key blocks
QC = N // 512       # 4 free-dim chunks of 512
HF = 1024           # q processed in halves
SCALE = float(D) ** -0.5  # 0.125

_CACHE = {}


def _build_program():
    import concourse.bass as bass
    import concourse.mybir as mybir
    import concourse.tile as tile
    from concourse import bacc
    from concourse.masks import make_identity

    f32 = mybir.dt.float32
    bf16 = mybir.dt.bfloat16
    Exp = mybir.ActivationFunctionType.Exp
    mult = mybir.AluOpType.mult
    add = mybir.AluOpType.add

    nc = bacc.Bacc()
    xT = nc.dram_tensor("xT", [C, N], bf16, kind="ExternalInput")
    wqkv = nc.dram_tensor("wqkv", [C, 3 * NH * D], bf16, kind="ExternalInput")
    wproj = nc.dram_tensor("wproj", [NH * D, C], bf16, kind="ExternalInput")
    temp = nc.dram_tensor("temp", [P, NH], f32, kind="ExternalInput")
    outT = nc.dram_tensor("outT", [C, N], f32, kind="ExternalOutput")
    rdram_s = nc.dram_tensor("rscratch_s", [2 * NH, HF], f32)  # denom rows
    rdram_r = nc.dram_tensor("rscratch_r", [2 * NH, HF], f32)  # recip rows

    mm = nc.tensor.matmul

    with tile.TileContext(nc) as tc:
        with (
            tc.tile_pool(name="const", bufs=1) as constp,
            tc.tile_pool(name="persist", bufs=1) as persist,
        ):
            # ---- setup: temperature diag masks (1 - t_h * I) ----------
            ident = constp.tile([P, P], f32, tag="ident")
            make_identity(nc, ident[:])
            tbc = constp.tile([P, NH], f32, tag="tbc")
            nc.sync.dma_start(tbc[:, :], temp[:, :])
            ntb = constp.tile([P, NH], f32, tag="ntb")
            nc.vector.tensor_scalar_mul(ntb[:, :], tbc[:, :], -1.0)
            masks = constp.tile([P, NH, P], f32, tag="masks")
            for h in range(NH):
                nc.vector.tensor_scalar(
                    masks[:, h, :], ident[:], ntb[:, h : h + 1], 1.0, mult, add
                )

            # persistent: qT/kT (head pairs stacked on partitions), v_aug
            qkT = persist.tile([P, 2 * NP, N], bf16, tag="qkT")  # 0-2 q, 3-5 k
            vaug = persist.tile([P, KB, NH, D + 1], bf16, tag="vaug")
            onesrc = constp.tile([P, KB * NH], f32, tag="onesrc")
            nc.vector.memset(onesrc[:], 1.0)
            nc.vector.tensor_copy(
                vaug[:, :, :, D : D + 1],
                onesrc[:].rearrange("p (a b c) -> p a b c", a=KB, b=NH),
            )
            wp = persist.tile([P, NH * D // P, C], bf16, tag="wp")  # [128,3,768]
            for g3 in range(NH * D // P):
                nc.sync.dma_start(wp[:, g3, :], wproj[g3 * P : (g3 + 1) * P, :])
            attnT = persist.tile([P, NP, N], bf16, tag="attnT")
            o01 = persist.tile([P, CT, N], f32, tag="o01")  # phase-3 partials

            # ---- phase 1 (prefix): v + pair-0's q/k only --------------
            # The remaining q/k projection groups are injected into the
            # phase-2 pipeline at intervals: each injection is a dense,
            # exp-independent PE burst that re-warms the HAM clock gate
            # (an ACT-bound steady state alone never has a 3.4us
            # contiguous-busy window, so the PE would stay cold forever).
            qin_cm = tc.tile_pool(name="qin", bufs=1)
            qin = qin_cm.__enter__()
            xts, wqs = [], []
            for t in range(CT):
                xti = qin.tile([P, N], bf16, tag=f"xt{t}", name=f"xt{t}")
                nc.sync.dma_start(xti[:], xT[t * P : (t + 1) * P, :])
                xts.append(xti)
                wqi = qin.tile([P, 3 * NH * D], bf16, tag=f"wq{t}", name=f"wq{t}")
                nc.sync.dma_start(wqi[:], wqkv[t * P : (t + 1) * P, :])
                wqs.append(wqi)

            with tc.tile_pool(name="psum1", bufs=2, space=bass.MemorySpace.PSUM) as psum1:

                def qk_group(g):
                    ps = psum1.tile([P, N], f32, tag="ps", name=f"ps{g}")
                    for t in range(CT):
                        for qc in range(QC):
                            mm(
                                ps[:, qc * 512 : (qc + 1) * 512],
                                wqs[t][:, g * P : (g + 1) * P],
                                xts[t][:, qc * 512 : (qc + 1) * 512],
                                start=(t == 0),
                                stop=(t == CT - 1),
                            )
                    nc.vector.tensor_copy(qkT[:, g, :], ps[:])

                def v_group(rb_i):
                    psv = psum1.tile([P, NH * D], f32, tag="ps", name=f"psv{rb_i}")
                    for t in range(CT):
                        mm(
                            psv[:],
                            xts[t][:, rb_i * P : (rb_i + 1) * P],
                            wqs[t][:, 2 * NH * D : 3 * NH * D],
                            start=(t == 0),
                            stop=(t == CT - 1),
                        )
                    nc.vector.tensor_copy(
                        vaug[:, rb_i, :, 0:D],
                        psv[:].rearrange("p (h d) -> p h d", h=NH),
                    )

                qk_group(0)
                for i in range(8):
                    v_group(i)
                qk_group(3)
                for i in range(8, 16):
                    v_group(i)

            # ---- phase 2: attention, one flat pipeline ----------------
            with (
                tc.tile_pool(name="pt", bufs=4) as ptp,
                tc.tile_pool(name="un", bufs=4) as unp,
                tc.tile_pool(name="rb", bufs=2) as rbp,
                tc.tile_pool(name="pst", bufs=2, space=bass.MemorySpace.PSUM) as pst,
                tc.tile_pool(name="pav", bufs=2, space=bass.MemorySpace.PSUM) as pav,
            ):
                def inject_qk(g, half):
                    # one [128, 1024] chunk of a q/k projection group,
                    # accumulated in an st-ring slot then copied to qkT
                    stq = pst.tile([P, HF], f32, tag="st", name=f"stq{g}_{half}")
                    for t in range(CT):
                        for qc in range(2):
                            cs = slice(qc * 512, (qc + 1) * 512)
                            xs = slice(half * HF + qc * 512,
                                       half * HF + (qc + 1) * 512)
                            mm(stq[:, cs], wqs[t][:, g * P : (g + 1) * P],
                               xts[t][:, xs], start=(t == 0), stop=(t == CT - 1))
                    nc.vector.tensor_copy(qkT[:, g, half * HF : (half + 1) * HF],
                                          stq[:])

                def inject_po01(m, half):
                    # phase-3 partial: accumulate pairs 0,1 (g3 = 0,1) of
                    # output-projection m-tile into an st-ring slot, stage
                    # the result in SBUF.  Only the g3 == 2 matmuls and an
                    # add remain for the tail.
                    pq = pst.tile([P, HF], f32, tag="st", name=f"pq{m}_{half}")
                    for g3 in range(2):
                        for qc in range(2):
                            cs = slice(qc * 512, (qc + 1) * 512)
                            acs = slice(half * HF + qc * 512,
                                        half * HF + (qc + 1) * 512)
                            mm(pq[:, cs], wp[:, g3, m * P : (m + 1) * P],
                               attnT[:, g3, acs],
                               start=(g3 == 0), stop=(g3 == 1))
                    nc.vector.tensor_copy(
                        o01[:, m, half * HF : (half + 1) * HF], pq[:]
                    )

                injections = {
                    (0, 5): lambda: inject_qk(1, 0),
                    (0, 11): lambda: inject_qk(1, 1),
                    (1, 3): lambda: inject_qk(4, 0),
                    (1, 9): lambda: inject_qk(4, 1),
                    (2, 5): lambda: inject_qk(2, 0),
                    (2, 11): lambda: inject_qk(2, 1),
                    (3, 3): lambda: inject_qk(5, 0),
                    (3, 9): lambda: inject_qk(5, 1),
                    # phase-3 partials over ready pairs keep the late
                    # blocks' PE dense (their qk injections are exhausted)
                    (3, 7): lambda: inject_po01(0, 0),
                    (3, 11): lambda: inject_po01(1, 0),
                    (3, 13): lambda: inject_po01(2, 0),
                    (3, 15): lambda: inject_po01(3, 0),
                    (4, 3): lambda: inject_po01(4, 0),
                    (4, 7): lambda: inject_po01(5, 0),
                    (4, 9): lambda: inject_po01(0, 1),
                    (4, 11): lambda: inject_po01(1, 1),
                    (4, 13): lambda: inject_po01(2, 1),
                    (4, 15): lambda: inject_po01(3, 1),
                    (5, 3): lambda: inject_po01(4, 1),
                    (5, 5): lambda: inject_po01(5, 1),
                }
                blocks = [(p, hf) for p in range(NP) for hf in range(2)]
                avs = [None] * len(blocks)
                prev = None      # (bi, kb, ptA, ptB)
                pending = []     # deferred normalize tails

                def emit_av(bi, kb, ptA, ptB):
                    p, hf = blocks[bi]
                    avA, avB = avs[bi]
                    for qc in range(2):
                        cs = slice(qc * 512, (qc + 1) * 512)
                        mm(avA[:, cs], vaug[:, kb, 2 * p, :], ptA[:, cs],
                           start=(kb == 0), stop=(kb == KB - 1))
                        mm(avB[:, cs], vaug[:, kb, 2 * p + 1, :], ptB[:, cs],
                           start=(kb == 0), stop=(kb == KB - 1))
                    if kb == KB - 1:
                        emit_norm_a(bi)

                def emit_norm_a(bi):
                    # copy av out of PSUM (frees the accumulator slot), ship
                    # the denominator row to DRAM; the rest is deferred.
                    p, hf = blocks[bi]
                    q0 = hf * HF
                    avA, avB = avs[bi]
                    for off, avX, h in ((0, avA, 2 * p), (D, avB, 2 * p + 1)):
                        un = unp.tile([D + 1, HF], f32, tag="un",
                                      name=f"un{bi}_{h}")
                        nc.vector.tensor_copy(un[:], avX[:])
                        ri = 2 * h + hf
                        nc.sync.dma_start(rdram_s[ri, :], un[D : D + 1, :])
                        pending.append(
                            lambda un=un, ri=ri, off=off, p=p, q0=q0:
                            emit_norm_b(un, ri, off, p, q0)
                        )

                def emit_norm_b(un, ri, off, p, q0):
                    rp = rbp.tile([8, P], f32, tag="rp", name=f"rp{ri}")
                    nc.sync.dma_start(
                        rp[0:8, :],
                        rdram_s[ri, :].rearrange("(a b) -> a b", a=8),
                    )
                    nc.vector.reciprocal(rp[0:8, :], rp[0:8, :])
                    nc.sync.dma_start(rdram_r[ri, :], rp[0:8, :])
                    rb = rbp.tile([D, HF], f32, tag="rb", name=f"rb{ri}")
                    nc.sync.dma_start(
                        rb[:],
                        rdram_r[ri : ri + 1, :].broadcast_to([D, HF]),
                    )
                    nc.vector.tensor_mul(
                        attnT[off : off + D, p, q0 : q0 + HF],
                        un[0:D, :],
                        rb[:],
                    )

                for bi, (p, hf) in enumerate(blocks):
                    q0 = hf * HF
                    hA, hB = 2 * p, 2 * p + 1
                    avs[bi] = (
                        pav.tile([D + 1, HF], f32, tag="av", name=f"avA{bi}"),
                        pav.tile([D + 1, HF], f32, tag="av", name=f"avB{bi}"),
                    )
                    for kb in range(KB):
                        if kb in (4, 5) and pending:
                            fn = pending.pop(0)
                            fn()
                        if (bi, kb) in injections:
                            injections[(bi, kb)]()
                        stA = pst.tile([P, HF], f32, tag="st", name=f"stA{bi}_{kb}")
                        stB = pst.tile([P, HF], f32, tag="st", name=f"stB{bi}_{kb}")
                        for qc in range(2):
                            cs = slice(qc * 512, (qc + 1) * 512)
                            qs = slice(q0 + qc * 512, q0 + (qc + 1) * 512)
                            ks = slice(kb * P, (kb + 1) * P)
                            mm(stA[:, cs], qkT[0:D, NP + p, ks],
                               qkT[0:D, p, qs], start=True, stop=True)
                            mm(stB[:, cs], qkT[D:P, NP + p, ks],
                               qkT[D:P, p, qs], start=True, stop=True)
                        if kb * P // HF == hf:
                            dcol = kb * P - q0
                            dsl = slice(dcol, dcol + P)
                            nc.vector.tensor_mul(
                                stA[:, dsl], stA[:, dsl], masks[:, hA, :]
                            )
                            nc.vector.tensor_mul(
                                stB[:, dsl], stB[:, dsl], masks[:, hB, :]
                            )
                        ptA = ptp.tile([P, HF], bf16, tag="pt", name=f"ptA{bi}_{kb}")
                        nc.scalar.activation(ptA[:], stA[:], Exp, scale=SCALE)
                        ptB = ptp.tile([P, HF], bf16, tag="pt", name=f"ptB{bi}_{kb}")
                        nc.scalar.activation(ptB[:], stB[:], Exp, scale=SCALE)
                        if prev is not None:
                            emit_av(*prev)
                        prev = (bi, kb, ptA, ptB)
                # drain: final AV flush + its normalize
                emit_av(*prev)
                for fn in pending:
                    fn()
                pending = []
            qin_cm.__exit__(None, None, None)

            # ---- phase 3 (tail): pair-2 contribution + staged partials
            with (
                tc.tile_pool(name="psum3", bufs=2, space=bass.MemorySpace.PSUM) as psum3,
                tc.tile_pool(name="ot", bufs=2) as otp,
            ):
                for m in range(CT):
                    po = psum3.tile([P, N], f32, tag="ps", name=f"po{m}")
                    for qc in range(QC):
                        cs = slice(qc * 512, (qc + 1) * 512)
                        mm(po[:, cs], wp[:, 2, m * P : (m + 1) * P],
                           attnT[:, 2, cs], start=True, stop=True)
                    ot = otp.tile([P, N], f32, tag="ot", name=f"ot{m}")
                    nc.vector.tensor_add(ot[:], po[:], o01[:, m, :])
                    nc.sync.dma_start(outT[m * P : (m + 1) * P, :], ot[:])

    if not nc.is_finalized():
        nc.finalize()
    return nc


def _get_program():
    if "nc" not in _CACHE:
        _CACHE["nc"] = _build_program()
    return _CACHE["nc"]


def _in_maps(x, w_qkv, w_proj, temperature):
    import ml_dtypes

    bf16 = ml_dtypes.bfloat16
    t = np.asarray(temperature, dtype=np.float32).reshape(H)
    maps = []
    xTs = {}
    for c in range(8):
        b, h0 = c // 2, NH * (c % 2)
        if b not in xTs:
            xTs[b] = np.ascontiguousarray(
                np.asarray(x[b], dtype=np.float32).T.astype(bf16)
            )
        cols = slice(D * h0, D * h0 + NH * D)
        wq = np.concatenate(
            [w_qkv[:, cols], w_qkv[:, C:][:, cols], w_qkv[:, 2 * C :][:, cols]],
            axis=1,
        )
        maps.append(
            {
                "xT": xTs[b],
                "wqkv": np.ascontiguousarray(wq).astype(bf16),
                "wproj": np.ascontiguousarray(
                    w_proj[D * h0 : D * h0 + NH * D, :]
                ).astype(bf16),
                "temp": np.ascontiguousarray(
                    np.broadcast_to(t[h0 : h0 + NH].reshape(1, NH), (P, NH))
                ),
            }
        )
    return maps


def _install_profile_hook():
    """The agent image's antenv lacks axon_hooks; synthesize it and register
    the ctypes NTFF hook so run_bass_kernel_spmd(trace=True) can profile."""
    import types, importlib

    if "antenv.axon_hooks" not in sys.modules:
        import antenv

        mod = types.ModuleType("antenv.axon_hooks")
        _state = {"hook": None}
        mod.set_axon_ntff_profile_hook = lambda h: _state.__setitem__("hook", h)
        mod.get_axon_ntff_profile_hook = lambda: _state["hook"]
        sys.modules["antenv.axon_hooks"] = mod
        antenv.axon_hooks = mod
    from antenv.axon_hooks import (
        get_axon_ntff_profile_hook,
        set_axon_ntff_profile_hook,
    )

    if get_axon_ntff_profile_hook() is None:
        tb = importlib.import_module("trn_agent_boot.trn_boot")
        hook = tb._ntff_profile_via_ctypes("/opt/axon/libaxon_pjrt.so")
        set_axon_ntff_profile_hook(hook)


def kernel(x, w_qkv, w_proj, b_proj, temperature, _trace=False):
    from concourse.bass_utils import run_bass_kernel_spmd

    if _trace:
        try:
            _install_profile_hook()
        except Exception as e:  # profiling is best-effort
            print(f"profile hook install failed: {e}")

    nc = _get_program()
    maps = _in_maps(
        np.asarray(x, np.float32),
        np.asarray(w_qkv, np.float32),
        np.asarray(w_proj, np.float32),
        np.asarray(temperature, np.float32),
    )
    res = run_bass_kernel_spmd(nc, maps, list(range(8)), trace=_trace)
    parts = [r["outT"] for r in res.results]
    bp = np.asarray(b_proj, np.float32)
    out = np.stack(
        [(parts[2 * b] + parts[2 * b + 1]).T + bp for b in range(B)]
    ).astype(np.float32)
    if _trace:
        _CACHE["last_result"] = res
    return out


# revision 36
# speedup vs baseline: 1.4364x; 1.0602x over previous
"""LocalitySelfAttention TRN2 kernel (v6: flat cross-block pipeline).

B=4, N=2048, C=768, H=12, D=64.  8 cores: core c -> batch c//2, heads
6*(c%2) .. 6*(c%2)+6 (6 contiguous heads = 3 pairs).  Each core computes its
heads' qkv projection, attention, and a partial output projection restricted
to its heads' 384 rows of w_proj.  Host sums the two partials per batch and
adds b_proj.

The whole phase-2 is ONE flat software pipeline over (pair, q-half, kb)
iterations with the AV matmuls lagging the score matmuls by exactly one
iteration, including across block boundaries.  This keeps the PE's strict
in-order queue free of semaphore waits (an AV never reaches the queue head
before its exp finished) and keeps PE activity dense enough that the HAM
clock gate stays at full rate.  Softmax normalization runs entirely off the
critical path: denominator rows round-trip through DRAM (reshaped [8,128]
so the DVE reciprocal runs on 8 partitions at free=128), with the dependent
DVE ops deferred half an iteration-block so their DMA waits are always
pre-satisfied when they reach the strict-FIFO DVE queue.
"""

import sys
import numpy as np

if "/opt/trn_rl_repo" not in sys.path:
    sys.path.insert(0, "/opt/trn_rl_repo")

B, N, C, H = 4, 2048, 768, 12
D = C // H          # 64
NH = 6              # heads per core
NP = NH // 2        # head pairs per core = 3
P = 128
CT = C // P         # 6 contraction tiles
KB = N // P         # 16 key blocks
QC = N // 512       # 4 free-dim chunks of 512
HF = 1024           # q processed in halves
SCALE = float(D) ** -0.5  # 0.125

_CACHE = {}


def _build_program():
    import concourse.bass as bass
    import concourse.mybir as mybir
    import concourse.tile as tile
    from concourse import bacc
    from concourse.masks import make_identity

    f32 = mybir.dt.float32
    bf16 = mybir.dt.bfloat16
    Exp = mybir.ActivationFunctionType.Exp
    mult = mybir.AluOpType.mult
    add = mybir.AluOpType.add

    nc = bacc.Bacc()
    xT = nc.dram_tensor("xT", [C, N], bf16, kind="ExternalInput")
    wqkv = nc.dram_tensor("wqkv", [C, 3 * NH * D], bf16, kind="ExternalInput")
    wproj = nc.dram_tensor("wproj", [NH * D, C], bf16, kind="ExternalInput")
    temp = nc.dram_tensor("temp", [P, NH], f32, kind="ExternalInput")
    outT = nc.dram_tensor("outT", [C, N], f32, kind="ExternalOutput")
    rdram_s = nc.dram_tensor("rscratch_s", [2 * NH, HF], f32)  # denom rows
    rdram_r = nc.dram_tensor("rscratch_r", [2 * NH, HF], f32)  # recip rows

    mm = nc.tensor.matmul

    with tile.TileContext(nc) as tc:
        with (
            tc.tile_pool(name="const", bufs=1) as constp,
            tc.tile_pool(name="persist", bufs=1) as persist,
        ):
            # ---- setup: temperature diag masks (1 - t_h * I) ----------
            ident = constp.tile([P, P], f32, tag="ident")
            make_identity(nc, ident[:])
            tbc = constp.tile([P, NH], f32, tag="tbc")
            nc.sync.dma_start(tbc[:, :], temp[:, :])
            ntb = constp.tile([P, NH], f32, tag="ntb")
            nc.vector.tensor_scalar_mul(ntb[:, :], tbc[:, :], -1.0)
            masks = constp.tile([P, NH, P], f32, tag="masks")
            for h in range(NH):
                nc.vector.tensor_scalar(
                    masks[:, h, :], ident[:], ntb[:, h : h + 1], 1.0, mult, add
                )

            # persistent: qT/kT (head pairs stacked on partitions), v_aug
            qkT = persist.tile([P, 2 * NP, N], bf16, tag="qkT")  # 0-2 q, 3-5 k
            vaug = persist.tile([P, KB, NH, D + 1], bf16, tag="vaug")
            onesrc = constp.tile([P, KB * NH], f32, tag="onesrc")
            nc.vector.memset(onesrc[:], 1.0)
            nc.vector.tensor_copy(
                vaug[:, :, :, D : D + 1],
                onesrc[:].rearrange("p (a b c) -> p a b c", a=KB, b=NH),
            )
            wp = persist.tile([P, NH * D // P, C], bf16, tag="wp")  # [128,3,768]
            for g3 in range(NH * D // P):
                nc.sync.dma_start(wp[:, g3, :], wproj[g3 * P : (g3 + 1) * P, :])
            attnT = persist.tile([P, NP, N], bf16, tag="attnT")
            o01 = persist.tile([P, CT, N], f32, tag="o01")  # phase-3 partials

            # ---- phase 1 (prefix): v + pair-0's q/k only --------------
            # The remaining q/k projection groups are injected into the
            # phase-2 pipeline at intervals: each injection is a dense,
            # exp-independent PE burst that re-warms the HAM clock gate
            # (an ACT-bound steady state alone never has a 3.4us
            # contiguous-busy window, so the PE would stay cold forever).
            qin_cm = tc.tile_pool(name="qin", bufs=1)
            qin = qin_cm.__enter__()
            xts, wqs = [], []
            for t in range(CT):
                xti = qin.tile([P, N], bf16, tag=f"xt{t}", name=f"xt{t}")
                nc.sync.dma_start(xti[:], xT[t * P : (t + 1) * P, :])
                xts.append(xti)
                wqi = qin.tile([P, 3 * NH * D], bf16, tag=f"wq{t}", name=f"wq{t}")
                nc.sync.dma_start(wqi[:], wqkv[t * P : (t + 1) * P, :])
                wqs.append(wqi)

            with tc.tile_pool(name="psum1", bufs=2, space=bass.MemorySpace.PSUM) as psum1:

                def qk_group(g):
                    ps = psum1.tile([P, N], f32, tag="ps", name=f"ps{g}")
                    for t in range(CT):
                        for qc in range(QC):
                            mm(
                                ps[:, qc * 512 : (qc + 1) * 512],
                                wqs[t][:, g * P : (g + 1) * P],
                                xts[t][:, qc * 512 : (qc + 1) * 512],
                                start=(t == 0),
                                stop=(t == CT - 1),
                            )
                    nc.vector.tensor_copy(qkT[:, g, :], ps[:])

                def v_group(rb_i):
                    psv = psum1.tile([P, NH * D], f32, tag="ps", name=f"psv{rb_i}")
                    for t in range(CT):
                        mm(
                            psv[:],
                            xts[t][:, rb_i * P : (rb_i + 1) * P],
                            wqs[t][:, 2 * NH * D : 3 * NH * D],
                            start=(t == 0),
                            stop=(t == CT - 1),
                        )
                    nc.vector.tensor_copy(
                        vaug[:, rb_i, :, 0:D],
                        psv[:].rearrange("p (h d) -> p h d", h=NH),
                    )

                qk_group(0)
                for i in range(8):
                    v_group(i)
                qk_group(3)
                for i in range(8, 16):
                    v_group(i)

            # ---- phase 2: attention, one flat pipeline ----------------
            with (
                tc.tile_pool(name="pt", bufs=4) as ptp,
                tc.tile_pool(name="un", bufs=4) as unp,
                tc.tile_pool(name="rb", bufs=2) as rbp,
                tc.tile_pool(name="pst", bufs=2, space=bass.MemorySpace.PSUM) as pst,
                tc.tile_pool(name="pav", bufs=2, space=bass.MemorySpace.PSUM) as pav,
            ):
                def inject_qk(g, half):
                    # one [128, 1024] chunk of a q/k projection group,
                    # accumulated in an st-ring slot then copied to qkT
                    stq = pst.tile([P, HF], f32, tag="st", name=f"stq{g}_{half}")
                    for t in range(CT):
                        for qc in range(2):
                            cs = slice(qc * 512, (qc + 1) * 512)
                            xs = slice(half * HF + qc * 512,
                                       half * HF + (qc + 1) * 512)
                            mm(stq[:, cs], wqs[t][:, g * P : (g + 1) * P],
                               xts[t][:, xs], start=(t == 0), stop=(t == CT - 1))
                    nc.vector.tensor_copy(qkT[:, g, half * HF : (half + 1) * HF],
                                          stq[:])

                def inject_po01(m, half):
                    # phase-3 partial: accumulate pairs 0,1 (g3 = 0,1) of
                    # output-projection m-tile into an st-ring slot, stage
                    # the result in SBUF.  Only the g3 == 2 matmuls and an
                    # add remain for the tail.
                    pq = pst.tile([P, HF], f32, tag="st", name=f"pq{m}_{half}")
                    for g3 in range(2):
                        for qc in range(2):
                            cs = slice(qc * 512, (qc + 1) * 512)
                            acs = slice(half * HF + qc * 512,
                                        half * HF + (qc + 1) * 512)
                            mm(pq[:, cs], wp[:, g3, m * P : (m + 1) * P],
                               attnT[:, g3, acs],
                               start=(g3 == 0), stop=(g3 == 1))
                    nc.vector.tensor_copy(
                        o01[:, m, half * HF : (half + 1) * HF], pq[:]
                    )

                def inject_po2h0(m):
                    # pair-2's contribution to m-tile, q-half 0 (its attnT
                    # is ready once block 4's normalize lands); completes
                    # the half-0 output, which ships to DRAM immediately.
                    pq2 = pst.tile([P, HF], f32, tag="st", name=f"pq2_{m}")
                    for qc in range(2):
                        cs = slice(qc * 512, (qc + 1) * 512)
                        mm(pq2[:, cs], wp[:, 2, m * P : (m + 1) * P],
                           attnT[:, 2, cs], start=True, stop=True)
                    nc.vector.tensor_add(o01[:, m, 0:HF],
                                         pq2[:], o01[:, m, 0:HF])
                    nc.sync.dma_start(outT[m * P : (m + 1) * P, 0:HF],
                                      o01[:, m, 0:HF])

                injections = {
                    (0, 5): lambda: inject_qk(1, 0),
                    (0, 11): lambda: inject_qk(1, 1),
                    (1, 3): lambda: inject_qk(4, 0),
                    (1, 9): lambda: inject_qk(4, 1),
                    (2, 5): lambda: inject_qk(2, 0),
                    (2, 11): lambda: inject_qk(2, 1),
                    (3, 3): lambda: inject_qk(5, 0),
                    (3, 9): lambda: inject_qk(5, 1),
                    # phase-3 partials over ready pairs keep the late
                    # blocks' PE dense (their qk injections are exhausted)
                    (3, 7): lambda: inject_po01(0, 0),
                    (3, 11): lambda: inject_po01(1, 0),
                    (3, 13): lambda: inject_po01(2, 0),
                    (3, 15): lambda: inject_po01(3, 0),
                    (4, 3): lambda: inject_po01(4, 0),
                    (4, 7): lambda: inject_po01(5, 0),
                    (4, 9): lambda: inject_po01(0, 1),
                    (4, 11): lambda: inject_po01(1, 1),
                    (4, 13): lambda: inject_po01(2, 1),
                    (4, 15): lambda: inject_po01(3, 1),
                    (5, 3): lambda: inject_po01(4, 1),
                    (5, 5): lambda: inject_po01(5, 1),
                    (5, 7): lambda: inject_po2h0(0),
                    (5, 9): lambda: inject_po2h0(1),
                    (5, 11): lambda: inject_po2h0(2),
                    (5, 13): lambda: inject_po2h0(3),
                    (5, 15): lambda: inject_po2h0(4),
                }
                blocks = [(p, hf) for p in range(NP) for hf in range(2)]
                avs = [None] * len(blocks)
                prev = None      # (bi, kb, ptA, ptB)
                pending = []     # deferred normalize tails

                def emit_av(bi, kb, ptA, ptB):
                    p, hf = blocks[bi]
                    avA, avB = avs[bi]
                    for qc in range(2):
                        cs = slice(qc * 512, (qc + 1) * 512)
                        mm(avA[:, cs], vaug[:, kb, 2 * p, :], ptA[:, cs],
                           start=(kb == 0), stop=(kb == KB - 1))
                        mm(avB[:, cs], vaug[:, kb, 2 * p + 1, :], ptB[:, cs],
                           start=(kb == 0), stop=(kb == KB - 1))
                    if kb == KB - 1:
                        emit_norm_a(bi)

                def emit_norm_a(bi):
                    # copy av out of PSUM (frees the accumulator slot), ship
                    # the denominator row to DRAM; the rest is deferred.
                    p, hf = blocks[bi]
                    q0 = hf * HF
                    avA, avB = avs[bi]
                    for off, avX, h in ((0, avA, 2 * p), (D, avB, 2 * p + 1)):
                        un = unp.tile([D + 1, HF], f32, tag="un",
                                      name=f"un{bi}_{h}")
                        nc.vector.tensor_copy(un[:], avX[:])
                        ri = 2 * h + hf
                        nc.sync.dma_start(rdram_s[ri, :], un[D : D + 1, :])
                        pending.append(
                            lambda un=un, ri=ri, off=off, p=p, q0=q0:
                            emit_norm_b(un, ri, off, p, q0)
                        )

                def emit_norm_b(un, ri, off, p, q0):
                    rp = rbp.tile([8, P], f32, tag="rp", name=f"rp{ri}")
                    nc.sync.dma_start(
                        rp[0:8, :],
                        rdram_s[ri, :].rearrange("(a b) -> a b", a=8),
                    )
                    nc.vector.reciprocal(rp[0:8, :], rp[0:8, :])
                    nc.sync.dma_start(rdram_r[ri, :], rp[0:8, :])
                    rb = rbp.tile([D, HF], f32, tag="rb", name=f"rb{ri}")
                    nc.sync.dma_start(
                        rb[:],
                        rdram_r[ri : ri + 1, :].broadcast_to([D, HF]),
                    )
                    nc.vector.tensor_mul(
                        attnT[off : off + D, p, q0 : q0 + HF],
                        un[0:D, :],
                        rb[:],
                    )

                for bi, (p, hf) in enumerate(blocks):
                    q0 = hf * HF
                    hA, hB = 2 * p, 2 * p + 1
                    avs[bi] = (
                        pav.tile([D + 1, HF], f32, tag="av", name=f"avA{bi}"),
                        pav.tile([D + 1, HF], f32, tag="av", name=f"avB{bi}"),
                    )
                    for kb in range(KB):
                        if kb in (4, 5) and pending:
                            fn = pending.pop(0)
                            fn()
                        if (bi, kb) in injections:
                            injections[(bi, kb)]()
                        stA = pst.tile([P, HF], f32, tag="st", name=f"stA{bi}_{kb}")
                        stB = pst.tile([P, HF], f32, tag="st", name=f"stB{bi}_{kb}")
                        for qc in range(2):
                            cs = slice(qc * 512, (qc + 1) * 512)
                            qs = slice(q0 + qc * 512, q0 + (qc + 1) * 512)
                            ks = slice(kb * P, (kb + 1) * P)
                            mm(stA[:, cs], qkT[0:D, NP + p, ks],
                               qkT[0:D, p, qs], start=True, stop=True)
                            mm(stB[:, cs], qkT[D:P, NP + p, ks],
                               qkT[D:P, p, qs], start=True, stop=True)
                        if kb * P // HF == hf:
                            dcol = kb * P - q0
                            dsl = slice(dcol, dcol + P)
                            nc.vector.tensor_mul(
                                stA[:, dsl], stA[:, dsl], masks[:, hA, :]
                            )
                            nc.vector.tensor_mul(
                                stB[:, dsl], stB[:, dsl], masks[:, hB, :]
                            )
                        ptA = ptp.tile([P, HF], bf16, tag="pt", name=f"ptA{bi}_{kb}")
                        nc.scalar.activation(ptA[:], stA[:], Exp, scale=SCALE)
                        ptB = ptp.tile([P, HF], bf16, tag="pt", name=f"ptB{bi}_{kb}")
                        nc.scalar.activation(ptB[:], stB[:], Exp, scale=SCALE)
                        if prev is not None:
                            emit_av(*prev)
                        prev = (bi, kb, ptA, ptB)
                # drain: final AV flush + normalize, last half-0 m-tile,
                # then the half-1 tail once block 5's normalize lands.
                emit_av(*prev)
                inject_po2h0(5)
                for fn in pending:
                    fn()
                pending = []
                for m in range(CT):
                    po = pst.tile([P, HF], f32, tag="st", name=f"po2h1_{m}")
                    for qc in range(2):
                        cs = slice(qc * 512, (qc + 1) * 512)
                        acs = slice(HF + qc * 512, HF + (qc + 1) * 512)
                        mm(po[:, cs], wp[:, 2, m * P : (m + 1) * P],
                           attnT[:, 2, acs], start=True, stop=True)
                    nc.vector.tensor_add(o01[:, m, HF:N],
                                         po[:], o01[:, m, HF:N])
                    nc.sync.dma_start(outT[m * P : (m + 1) * P, HF:N],
                                      o01[:, m, HF:N])
            qin_cm.__exit__(None, None, None)

    if not nc.is_finalized():
        nc.finalize()
    return nc


def _get_program():
    if "nc" not in _CACHE:
        _CACHE["nc"] = _build_program()
    return _CACHE["nc"]


def _in_maps(x, w_qkv, w_proj, temperature):
    import ml_dtypes

    bf16 = ml_dtypes.bfloat16
    t = np.asarray(temperature, dtype=np.float32).reshape(H)
    maps = []
    xTs = {}
    for c in range(8):
        b, h0 = c // 2, NH * (c % 2)
        if b not in xTs:
            xTs[b] = np.ascontiguousarray(
                np.asarray(x[b], dtype=np.float32).T.astype(bf16)
            )
        cols = slice(D * h0, D * h0 + NH * D)
        wq = np.concatenate(
            [w_qkv[:, cols], w_qkv[:, C:][:, cols], w_qkv[:, 2 * C :][:, cols]],
            axis=1,
        )
        maps.append(
            {
                "xT": xTs[b],
                "wqkv": np.ascontiguousarray(wq).astype(bf16),
                "wproj": np.ascontiguousarray(
                    w_proj[D * h0 : D * h0 + NH * D, :]
                ).astype(bf16),
                "temp": np.ascontiguousarray(
                    np.broadcast_to(t[h0 : h0 + NH].reshape(1, NH), (P, NH))
                ),
            }
        )
    return maps


def _install_profile_hook():
    """The agent image's antenv lacks axon_hooks; synthesize it and register
    the ctypes NTFF hook so run_bass_kernel_spmd(trace=True) can profile."""
    import types, importlib

    if "antenv.axon_hooks" not in sys.modules:
        import antenv

        mod = types.ModuleType("antenv.axon_hooks")
        _state = {"hook": None}
        mod.set_axon_ntff_profile_hook = lambda h: _state.__setitem__("hook", h)
        mod.get_axon_ntff_profile_hook = lambda: _state["hook"]
        sys.modules["antenv.axon_hooks"] = mod
        antenv.axon_hooks = mod
    from antenv.axon_hooks import (
        get_axon_ntff_profile_hook,
        set_axon_ntff_profile_hook,
    )

    if get_axon_ntff_profile_hook() is None:
        tb = importlib.import_module("trn_agent_boot.trn_boot")
        hook = tb._ntff_profile_via_ctypes("/opt/axon/libaxon_pjrt.so")
        set_axon_ntff_profile_hook(hook)


def kernel(x, w_qkv, w_proj, b_proj, temperature, _trace=False):
    from concourse.bass_utils import run_bass_kernel_spmd

    if _trace:
        try:
            _install_profile_hook()
        except Exception as e:  # profiling is best-effort
            print(f"profile hook install failed: {e}")

    nc = _get_program()
    maps = _in_maps(
        np.asarray(x, np.float32),
        np.asarray(w_qkv, np.float32),
        np.asarray(w_proj, np.float32),
        np.asarray(temperature, np.float32),
    )
    res = run_bass_kernel_spmd(nc, maps, list(range(8)), trace=_trace)
    parts = [r["outT"] for r in res.results]
    bp = np.asarray(b_proj, np.float32)
    out = np.stack(
        [(parts[2 * b] + parts[2 * b + 1]).T + bp for b in range(B)]
    ).astype(np.float32)
    if _trace:
        _CACHE["last_result"] = res
    return out


# revision 40
# speedup vs baseline: 1.4405x; 1.0029x over previous
"""LocalitySelfAttention TRN2 kernel (v6: flat cross-block pipeline).

B=4, N=2048, C=768, H=12, D=64.  8 cores: core c -> batch c//2, heads
6*(c%2) .. 6*(c%2)+6 (6 contiguous heads = 3 pairs).  Each core computes its
heads' qkv projection, attention, and a partial output projection restricted
to its heads' 384 rows of w_proj.  Host sums the two partials per batch and
adds b_proj.

The whole phase-2 is ONE flat software pipeline over (pair, q-half, kb)
iterations with the AV matmuls lagging the score matmuls by exactly one
iteration, including across block boundaries.  This keeps the PE's strict
in-order queue free of semaphore waits (an AV never reaches the queue head
before its exp finished) and keeps PE activity dense enough that the HAM
clock gate stays at full rate.  Softmax normalization runs entirely off the
critical path: denominator rows round-trip through DRAM (reshaped [8,128]
so the DVE reciprocal runs on 8 partitions at free=128), with the dependent
DVE ops deferred half an iteration-block so their DMA waits are always
pre-satisfied when they reach the strict-FIFO DVE queue.
"""

import sys
import numpy as np

if "/opt/trn_rl_repo" not in sys.path:
    sys.path.insert(0, "/opt/trn_rl_repo")

B, N, C, H = 4, 2048, 768, 12
D = C // H          # 64
NH = 6              # heads per core
NP = NH // 2        # head pairs per core = 3
P = 128
CT = C // P         # 6 contraction tiles
KB = N // P         # 16 key blocks
QC = N // 512       # 4 free-dim chunks of 512
HF = 1024           # q processed in halves
SCALE = float(D) ** -0.5  # 0.125

_CACHE = {}


def _build_program():
    import concourse.bass as bass
    import concourse.mybir as mybir
    import concourse.tile as tile
    from concourse import bacc
    from concourse.masks import make_identity

    f32 = mybir.dt.float32
    bf16 = mybir.dt.bfloat16
    Exp = mybir.ActivationFunctionType.Exp
    mult = mybir.AluOpType.mult
    add = mybir.AluOpType.add

    nc = bacc.Bacc()
    xT = nc.dram_tensor("xT", [C, N], bf16, kind="ExternalInput")
    wqkv = nc.dram_tensor("wqkv", [C, 3 * NH * D], bf16, kind="ExternalInput")
    wproj = nc.dram_tensor("wproj", [NH * D, C], bf16, kind="ExternalInput")
    temp = nc.dram_tensor("temp", [P, NH], f32, kind="ExternalInput")
    outT = nc.dram_tensor("outT", [C, N], f32, kind="ExternalOutput")
    rdram_s = nc.dram_tensor("rscratch_s", [2 * NH, HF], f32)  # denom rows
    rdram_r = nc.dram_tensor("rscratch_r", [2 * NH, HF], f32)  # recip rows

    mm = nc.tensor.matmul

    with tile.TileContext(nc) as tc:
        with (
            tc.tile_pool(name="const", bufs=1) as constp,
            tc.tile_pool(name="persist", bufs=1) as persist,
        ):
            # ---- setup: temperature diag masks (1 - t_h * I) ----------
            ident = constp.tile([P, P], f32, tag="ident")
            make_identity(nc, ident[:])
            tbc = constp.tile([P, NH], f32, tag="tbc")
            nc.sync.dma_start(tbc[:, :], temp[:, :])
            ntb = constp.tile([P, NH], f32, tag="ntb")
            nc.vector.tensor_scalar_mul(ntb[:, :], tbc[:, :], -1.0)
            masks = constp.tile([P, NH, P], f32, tag="masks")
            for h in range(NH):
                nc.vector.tensor_scalar(
                    masks[:, h, :], ident[:], ntb[:, h : h + 1], 1.0, mult, add
                )

            # persistent: qT/kT (head pairs stacked on partitions), v_aug
            qkT = persist.tile([P, 2 * NP, N], bf16, tag="qkT")  # 0-2 q, 3-5 k
            vaug = persist.tile([P, KB, NH, D + 1], bf16, tag="vaug")
            onesrc = constp.tile([P, KB * NH], f32, tag="onesrc")
            nc.vector.memset(onesrc[:], 1.0)
            nc.vector.tensor_copy(
                vaug[:, :, :, D : D + 1],
                onesrc[:].rearrange("p (a b c) -> p a b c", a=KB, b=NH),
            )
            wp = persist.tile([P, NH * D // P, C], bf16, tag="wp")  # [128,3,768]
            for g3 in range(NH * D // P):
                nc.sync.dma_start(wp[:, g3, :], wproj[g3 * P : (g3 + 1) * P, :])
            attnT = persist.tile([P, NP, N], bf16, tag="attnT")
            o01 = persist.tile([P, CT, N], f32, tag="o01")  # phase-3 partials

            # ---- phase 1 (prefix): v + pair-0's q/k only --------------
            # The remaining q/k projection groups are injected into the
            # phase-2 pipeline at intervals: each injection is a dense,
            # exp-independent PE burst that re-warms the HAM clock gate
            # (an ACT-bound steady state alone never has a 3.4us
            # contiguous-busy window, so the PE would stay cold forever).
            qin_cm = tc.tile_pool(name="qin", bufs=1)
            qin = qin_cm.__enter__()
            xts, wqs = [], []
            for t in range(CT):
                xti = qin.tile([P, N], bf16, tag=f"xt{t}", name=f"xt{t}")
                nc.sync.dma_start(xti[:], xT[t * P : (t + 1) * P, :])
                xts.append(xti)
                wqi = qin.tile([P, 3 * NH * D], bf16, tag=f"wq{t}", name=f"wq{t}")
                nc.sync.dma_start(wqi[:], wqkv[t * P : (t + 1) * P, :])
                wqs.append(wqi)

            with tc.tile_pool(name="psum1", bufs=2, space=bass.MemorySpace.PSUM) as psum1:

                def qk_group(g):
                    ps = psum1.tile([P, N], f32, tag="ps", name=f"ps{g}")
                    for t in range(CT):
                        for qc in range(QC):
                            mm(
                                ps[:, qc * 512 : (qc + 1) * 512],
                                wqs[t][:, g * P : (g + 1) * P],
                                xts[t][:, qc * 512 : (qc + 1) * 512],
                                start=(t == 0),
                                stop=(t == CT - 1),
                            )
                    nc.vector.tensor_copy(qkT[:, g, :], ps[:])

                def v_group(rb_i):
                    psv = psum1.tile([P, NH * D], f32, tag="ps", name=f"psv{rb_i}")
                    for t in range(CT):
                        mm(
                            psv[:],
                            xts[t][:, rb_i * P : (rb_i + 1) * P],
                            wqs[t][:, 2 * NH * D : 3 * NH * D],
                            start=(t == 0),
                            stop=(t == CT - 1),
                        )
                    nc.vector.tensor_copy(
                        vaug[:, rb_i, :, 0:D],
                        psv[:].rearrange("p (h d) -> p h d", h=NH),
                    )

                # q/k of pair 0 first: the first exp fires ~20us in; the
                # v groups complete just ahead of their AV consumers.
                qk_group(0)
                qk_group(3)
                for i in range(16):
                    v_group(i)

            # ---- phase 2: attention, one flat pipeline ----------------
            with (
                tc.tile_pool(name="pt", bufs=4) as ptp,
                tc.tile_pool(name="un", bufs=4) as unp,
                tc.tile_pool(name="rb", bufs=2) as rbp,
                tc.tile_pool(name="pst", bufs=2, space=bass.MemorySpace.PSUM) as pst,
                tc.tile_pool(name="pav", bufs=2, space=bass.MemorySpace.PSUM) as pav,
            ):
                def inject_qk(g, qtr):
                    # one [128, 512] quarter of a q/k projection group,
                    # accumulated in an st-ring slot then copied to qkT.
                    # Quarter-size keeps the slot hold ~1.3us so the exp
                    # pipeline's double buffering barely degrades.
                    stq = pst.tile([P, 512], f32, tag="st", name=f"stq{g}_{qtr}")
                    xs = slice(qtr * 512, (qtr + 1) * 512)
                    for t in range(CT):
                        mm(stq[:], wqs[t][:, g * P : (g + 1) * P],
                           xts[t][:, xs], start=(t == 0), stop=(t == CT - 1))
                    nc.vector.tensor_copy(qkT[:, g, xs], stq[:])

                def inject_po01(m, half):
                    # phase-3 partial: accumulate pairs 0,1 (g3 = 0,1) of
                    # output-projection m-tile into an st-ring slot, stage
                    # the result in SBUF.  Only the g3 == 2 matmuls and an
                    # add remain for the tail.
                    pq = pst.tile([P, HF], f32, tag="st", name=f"pq{m}_{half}")
                    for g3 in range(2):
                        for qc in range(2):
                            cs = slice(qc * 512, (qc + 1) * 512)
                            acs = slice(half * HF + qc * 512,
                                        half * HF + (qc + 1) * 512)
                            mm(pq[:, cs], wp[:, g3, m * P : (m + 1) * P],
                               attnT[:, g3, acs],
                               start=(g3 == 0), stop=(g3 == 1))
                    nc.vector.tensor_copy(
                        o01[:, m, half * HF : (half + 1) * HF], pq[:]
                    )

                def inject_po2h0(m):
                    # pair-2's contribution to m-tile, q-half 0 (its attnT
                    # is ready once block 4's normalize lands); completes
                    # the half-0 output, which ships to DRAM immediately.
                    pq2 = pst.tile([P, HF], f32, tag="st", name=f"pq2_{m}")
                    for qc in range(2):
                        cs = slice(qc * 512, (qc + 1) * 512)
                        mm(pq2[:, cs], wp[:, 2, m * P : (m + 1) * P],
                           attnT[:, 2, cs], start=True, stop=True)
                    nc.vector.tensor_add(o01[:, m, 0:HF],
                                         pq2[:], o01[:, m, 0:HF])
                    nc.sync.dma_start(outT[m * P : (m + 1) * P, 0:HF],
                                      o01[:, m, 0:HF])

                injections = {}
                for i, g in enumerate((1, 4, 2)):
                    for qtr in range(4):
                        injections[(i, (3, 5, 9, 11)[qtr])] = (
                            lambda g=g, q=qtr: inject_qk(g, q)
                        )
                for qtr in range(4):
                    injections[(3, (3, 5, 9, 11)[qtr])] = (
                        lambda q=qtr: inject_qk(5, q)
                    )
                # phase-3 partials over ready pairs keep the late blocks'
                # PE dense once the qk injections are exhausted
                po01_sched = [
                    (3, 7, 0, 0), (3, 13, 1, 0), (3, 15, 2, 0),
                    (4, 3, 3, 0), (4, 5, 4, 0), (4, 7, 5, 0),
                    (4, 9, 0, 1), (4, 11, 1, 1), (4, 13, 2, 1), (4, 15, 3, 1),
                    (5, 3, 4, 1), (5, 5, 5, 1),
                ]
                for bi_, kb_, m_, h_ in po01_sched:
                    injections[(bi_, kb_)] = (
                        lambda m=m_, h=h_: inject_po01(m, h)
                    )
                for i, kb_ in enumerate((7, 9, 11, 13, 15)):
                    injections[(5, kb_)] = lambda m=i: inject_po2h0(m)
                blocks = [(p, hf) for p in range(NP) for hf in range(2)]
                avs = [None] * len(blocks)
                prev = None      # (bi, kb, ptA, ptB)
                pending = []     # deferred normalize tails

                def emit_av(bi, kb, ptA, ptB):
                    p, hf = blocks[bi]
                    avA, avB = avs[bi]
                    for qc in range(2):
                        cs = slice(qc * 512, (qc + 1) * 512)
                        mm(avA[:, cs], vaug[:, kb, 2 * p, :], ptA[:, cs],
                           start=(kb == 0), stop=(kb == KB - 1))
                        mm(avB[:, cs], vaug[:, kb, 2 * p + 1, :], ptB[:, cs],
                           start=(kb == 0), stop=(kb == KB - 1))
                    if kb == KB - 1:
                        emit_norm_a(bi)

                def emit_norm_a(bi):
                    # copy av out of PSUM (frees the accumulator slot), ship
                    # the denominator row to DRAM; the rest is deferred.
                    p, hf = blocks[bi]
                    q0 = hf * HF
                    avA, avB = avs[bi]
                    for off, avX, h in ((0, avA, 2 * p), (D, avB, 2 * p + 1)):
                        un = unp.tile([D + 1, HF], f32, tag="un",
                                      name=f"un{bi}_{h}")
                        nc.vector.tensor_copy(un[:], avX[:])
                        ri = 2 * h + hf
                        nc.sync.dma_start(rdram_s[ri, :], un[D : D + 1, :])
                        pending.append(
                            lambda un=un, ri=ri, off=off, p=p, q0=q0:
                            emit_norm_b(un, ri, off, p, q0)
                        )

                def emit_norm_b(un, ri, off, p, q0):
                    rp = rbp.tile([8, P], f32, tag="rp", name=f"rp{ri}")
                    nc.sync.dma_start(
                        rp[0:8, :],
                        rdram_s[ri, :].rearrange("(a b) -> a b", a=8),
                    )
                    nc.vector.reciprocal(rp[0:8, :], rp[0:8, :])
                    nc.sync.dma_start(rdram_r[ri, :], rp[0:8, :])
                    rb = rbp.tile([D, HF], f32, tag="rb", name=f"rb{ri}")
                    nc.sync.dma_start(
                        rb[:],
                        rdram_r[ri : ri + 1, :].broadcast_to([D, HF]),
                    )
                    nc.vector.tensor_mul(
                        attnT[off : off + D, p, q0 : q0 + HF],
                        un[0:D, :],
                        rb[:],
                    )

                for bi, (p, hf) in enumerate(blocks):
                    q0 = hf * HF
                    hA, hB = 2 * p, 2 * p + 1
                    avs[bi] = (
                        pav.tile([D + 1, HF], f32, tag="av", name=f"avA{bi}"),
                        pav.tile([D + 1, HF], f32, tag="av", name=f"avB{bi}"),
                    )
                    for kb in range(KB):
                        if kb in (4, 5) and pending:
                            fn = pending.pop(0)
                            fn()
                        if (bi, kb) in injections:
                            injections[(bi, kb)]()
                        stA = pst.tile([P, HF], f32, tag="st", name=f"stA{bi}_{kb}")
                        stB = pst.tile([P, HF], f32, tag="st", name=f"stB{bi}_{kb}")
                        for qc in range(2):
                            cs = slice(qc * 512, (qc + 1) * 512)
                            qs = slice(q0 + qc * 512, q0 + (qc + 1) * 512)
                            ks = slice(kb * P, (kb + 1) * P)
                            mm(stA[:, cs], qkT[0:D, NP + p, ks],
                               qkT[0:D, p, qs], start=True, stop=True)
                            mm(stB[:, cs], qkT[D:P, NP + p, ks],
                               qkT[D:P, p, qs], start=True, stop=True)
                        if kb * P // HF == hf:
                            dcol = kb * P - q0
                            dsl = slice(dcol, dcol + P)
                            nc.vector.tensor_mul(
                                stA[:, dsl], stA[:, dsl], masks[:, hA, :]
                            )
                            nc.vector.tensor_mul(
                                stB[:, dsl], stB[:, dsl], masks[:, hB, :]
                            )
                        ptA = ptp.tile([P, HF], bf16, tag="pt", name=f"ptA{bi}_{kb}")
                        nc.scalar.activation(ptA[:], stA[:], Exp, scale=SCALE)
                        ptB = ptp.tile([P, HF], bf16, tag="pt", name=f"ptB{bi}_{kb}")
                        nc.scalar.activation(ptB[:], stB[:], Exp, scale=SCALE)
                        if prev is not None:
                            emit_av(*prev)
                        prev = (bi, kb, ptA, ptB)
                # drain: final AV flush + normalize (its small DMAs go
                # first so they aren't queued behind big output DMAs),
                # last half-0 m-tile, then the half-1 tail.
                emit_av(*prev)
                for fn in pending:
                    fn()
                pending = []
                inject_po2h0(5)
                for m in range(CT):
                    po = pst.tile([P, HF], f32, tag="st", name=f"po2h1_{m}")
                    for qc in range(2):
                        cs = slice(qc * 512, (qc + 1) * 512)
                        acs = slice(HF + qc * 512, HF + (qc + 1) * 512)
                        mm(po[:, cs], wp[:, 2, m * P : (m + 1) * P],
                           attnT[:, 2, acs], start=True, stop=True)
                    nc.vector.tensor_add(o01[:, m, HF:N],
                                         po[:], o01[:, m, HF:N])
                    nc.sync.dma_start(outT[m * P : (m + 1) * P, HF:N],
                                      o01[:, m, HF:N])
            qin_cm.__exit__(None, None, None)

    if not nc.is_finalized():
        nc.finalize()
    return nc


def _get_program():
    if "nc" not in _CACHE:
        _CACHE["nc"] = _build_program()
    return _CACHE["nc"]


def _in_maps(x, w_qkv, w_proj, temperature):
    import ml_dtypes

    bf16 = ml_dtypes.bfloat16
    t = np.asarray(temperature, dtype=np.float32).reshape(H)
    maps = []
    xTs = {}
    for c in range(8):
        b, h0 = c // 2, NH * (c % 2)
        if b not in xTs:
            xTs[b] = np.ascontiguousarray(
                np.asarray(x[b], dtype=np.float32).T.astype(bf16)
            )
        cols = slice(D * h0, D * h0 + NH * D)
        wq = np.concatenate(
            [w_qkv[:, cols], w_qkv[:, C:][:, cols], w_qkv[:, 2 * C :][:, cols]],
            axis=1,
        )
        maps.append(
            {
                "xT": xTs[b],
                "wqkv": np.ascontiguousarray(wq).astype(bf16),
                "wproj": np.ascontiguousarray(
                    w_proj[D * h0 : D * h0 + NH * D, :]
                ).astype(bf16),
                "temp": np.ascontiguousarray(
                    np.broadcast_to(t[h0 : h0 + NH].reshape(1, NH), (P, NH))
                ),
            }
        )
    return maps


def _install_profile_hook():
    """The agent image's antenv lacks axon_hooks; synthesize it and register
    the ctypes NTFF hook so run_bass_kernel_spmd(trace=True) can profile."""
    import types, importlib

    if "antenv.axon_hooks" not in sys.modules:
        import antenv

        mod = types.ModuleType("antenv.axon_hooks")
        _state = {"hook": None}
        mod.set_axon_ntff_profile_hook = lambda h: _state.__setitem__("hook", h)
        mod.get_axon_ntff_profile_hook = lambda: _state["hook"]
        sys.modules["antenv.axon_hooks"] = mod
        antenv.axon_hooks = mod
    from antenv.axon_hooks import (
        get_axon_ntff_profile_hook,
        set_axon_ntff_profile_hook,
    )

    if get_axon_ntff_profile_hook() is None:
        tb = importlib.import_module("trn_agent_boot.trn_boot")
        hook = tb._ntff_profile_via_ctypes("/opt/axon/libaxon_pjrt.so")
        set_axon_ntff_profile_hook(hook)


def kernel(x, w_qkv, w_proj, b_proj, temperature, _trace=False):
    from concourse.bass_utils import run_bass_kernel_spmd

    if _trace:
        try:
            _install_profile_hook()
        except Exception as e:  # profiling is best-effort
            print(f"profile hook install failed: {e}")

    nc = _get_program()
    maps = _in_maps(
        np.asarray(x, np.float32),
        np.asarray(w_qkv, np.float32),
        np.asarray(w_proj, np.float32),
        np.asarray(temperature, np.float32),
    )
    res = run_bass_kernel_spmd(nc, maps, list(range(8)), trace=_trace)
    parts = [r["outT"] for r in res.results]
    bp = np.asarray(b_proj, np.float32)
    out = np.stack(
        [(parts[2 * b] + parts[2 * b + 1]).T + bp for b in range(B)]
    ).astype(np.float32)
    if _trace:
        _CACHE["last_result"] = res
    return out


# revision 43
# speedup vs baseline: 1.4567x; 1.0113x over previous
"""LocalitySelfAttention TRN2 kernel (v6: flat cross-block pipeline).

B=4, N=2048, C=768, H=12, D=64.  8 cores: core c -> batch c//2, heads
6*(c%2) .. 6*(c%2)+6 (6 contiguous heads = 3 pairs).  Each core computes its
heads' qkv projection, attention, and a partial output projection restricted
to its heads' 384 rows of w_proj.  Host sums the two partials per batch and
adds b_proj.

The whole phase-2 is ONE flat software pipeline over (pair, q-half, kb)
iterations with the AV matmuls lagging the score matmuls by exactly one
iteration, including across block boundaries.  This keeps the PE's strict
in-order queue free of semaphore waits (an AV never reaches the queue head
before its exp finished) and keeps PE activity dense enough that the HAM
clock gate stays at full rate.  Softmax normalization runs entirely off the
critical path: denominator rows round-trip through DRAM (reshaped [8,128]
so the DVE reciprocal runs on 8 partitions at free=128), with the dependent
DVE ops deferred half an iteration-block so their DMA waits are always
pre-satisfied when they reach the strict-FIFO DVE queue.
"""

import sys
import numpy as np

if "/opt/trn_rl_repo" not in sys.path:
    sys.path.insert(0, "/opt/trn_rl_repo")

B, N, C, H = 4, 2048, 768, 12
D = C // H          # 64
NH = 6              # heads per core
NP = NH // 2        # head pairs per core = 3
P = 128
CT = C // P         # 6 contraction tiles
KB = N // P         # 16 key blocks
QC = N // 512       # 4 free-dim chunks of 512
HF = 1024           # q processed in halves
SCALE = float(D) ** -0.5  # 0.125

_CACHE = {}


def _build_program():
    import concourse.bass as bass
    import concourse.mybir as mybir
    import concourse.tile as tile
    from concourse import bacc
    from concourse.masks import make_identity

    f32 = mybir.dt.float32
    bf16 = mybir.dt.bfloat16
    Exp = mybir.ActivationFunctionType.Exp
    mult = mybir.AluOpType.mult
    add = mybir.AluOpType.add

    nc = bacc.Bacc()
    xT = nc.dram_tensor("xT", [C, N], bf16, kind="ExternalInput")
    wqkv = nc.dram_tensor("wqkv", [C, 3 * NH * D], bf16, kind="ExternalInput")
    wproj = nc.dram_tensor("wproj", [NH * D, C], bf16, kind="ExternalInput")
    temp = nc.dram_tensor("temp", [P, NH], f32, kind="ExternalInput")
    outT = nc.dram_tensor("outT", [C, N], f32, kind="ExternalOutput")
    rdram_s = nc.dram_tensor("rscratch_s", [2 * NH, HF], f32)  # denom rows
    rdram_r = nc.dram_tensor("rscratch_r", [2 * NH, HF], f32)  # recip rows

    mm = nc.tensor.matmul

    with tile.TileContext(nc) as tc:
        with (
            tc.tile_pool(name="const", bufs=1) as constp,
            tc.tile_pool(name="persist", bufs=1) as persist,
        ):
            # ---- setup: temperature diag masks (1 - t_h * I) ----------
            ident = constp.tile([P, P], f32, tag="ident")
            make_identity(nc, ident[:])
            tbc = constp.tile([P, NH], f32, tag="tbc")
            nc.sync.dma_start(tbc[:, :], temp[:, :])
            ntb = constp.tile([P, NH], f32, tag="ntb")
            nc.vector.tensor_scalar_mul(ntb[:, :], tbc[:, :], -1.0)
            masks = constp.tile([P, NH, P], f32, tag="masks")
            for h in range(NH):
                nc.vector.tensor_scalar(
                    masks[:, h, :], ident[:], ntb[:, h : h + 1], 1.0, mult, add
                )

            # persistent: qT/kT (head pairs stacked on partitions), v_aug
            qkT = persist.tile([P, 2 * NP, N], bf16, tag="qkT")  # 0-2 q, 3-5 k
            vaug = persist.tile([P, KB, NH, D + 1], bf16, tag="vaug")
            onesrc = constp.tile([P, KB * NH], f32, tag="onesrc")
            nc.vector.memset(onesrc[:], 1.0)
            nc.vector.tensor_copy(
                vaug[:, :, :, D : D + 1],
                onesrc[:].rearrange("p (a b c) -> p a b c", a=KB, b=NH),
            )
            wp = persist.tile([P, NH * D // P, C], bf16, tag="wp")  # [128,3,768]
            for g3 in range(NH * D // P):
                nc.sync.dma_start(wp[:, g3, :], wproj[g3 * P : (g3 + 1) * P, :])
            attnT = persist.tile([P, NP, N], bf16, tag="attnT")
            o01 = persist.tile([P, CT, N], f32, tag="o01")  # phase-3 partials

            # ---- phase 1 (prefix): v + pair-0's q/k only --------------
            # The remaining q/k projection groups are injected into the
            # phase-2 pipeline at intervals: each injection is a dense,
            # exp-independent PE burst that re-warms the HAM clock gate
            # (an ACT-bound steady state alone never has a 3.4us
            # contiguous-busy window, so the PE would stay cold forever).
            qin_cm = tc.tile_pool(name="qin", bufs=1)
            qin = qin_cm.__enter__()
            xts, wqs = [], []
            for t in range(CT):
                xti = qin.tile([P, N], bf16, tag=f"xt{t}", name=f"xt{t}")
                nc.sync.dma_start(xti[:], xT[t * P : (t + 1) * P, :])
                xts.append(xti)
                wqi = qin.tile([P, 3 * NH * D], bf16, tag=f"wq{t}", name=f"wq{t}")
                nc.sync.dma_start(wqi[:], wqkv[t * P : (t + 1) * P, :])
                wqs.append(wqi)

            with tc.tile_pool(name="psum1", bufs=2, space=bass.MemorySpace.PSUM) as psum1:

                def qk_group(g):
                    ps = psum1.tile([P, N], f32, tag="ps", name=f"ps{g}")
                    for t in range(CT):
                        for qc in range(QC):
                            mm(
                                ps[:, qc * 512 : (qc + 1) * 512],
                                wqs[t][:, g * P : (g + 1) * P],
                                xts[t][:, qc * 512 : (qc + 1) * 512],
                                start=(t == 0),
                                stop=(t == CT - 1),
                            )
                    nc.vector.tensor_copy(qkT[:, g, :], ps[:])

                def v_group(rb_i):
                    psv = psum1.tile([P, NH * D], f32, tag="ps", name=f"psv{rb_i}")
                    for t in range(CT):
                        mm(
                            psv[:],
                            xts[t][:, rb_i * P : (rb_i + 1) * P],
                            wqs[t][:, 2 * NH * D : 3 * NH * D],
                            start=(t == 0),
                            stop=(t == CT - 1),
                        )
                    nc.vector.tensor_copy(
                        vaug[:, rb_i, :, 0:D],
                        psv[:].rearrange("p (h d) -> p h d", h=NH),
                    )

                # q/k of pair 0 and the first few v groups only: the PE
                # queue is strictly in-order, so anything emitted here
                # delays the first exp.  v4-v15 are injected into early
                # block-0 iterations, just ahead of their AV consumers.
                qk_group(0)
                qk_group(3)
                for i in range(4):
                    v_group(i)

            # ---- phase 2: attention, one flat pipeline ----------------
            with (
                tc.tile_pool(name="pt", bufs=4) as ptp,
                tc.tile_pool(name="un", bufs=4) as unp,
                tc.tile_pool(name="rb", bufs=2) as rbp,
                tc.tile_pool(name="pst", bufs=2, space=bass.MemorySpace.PSUM) as pst,
                tc.tile_pool(name="pav", bufs=2, space=bass.MemorySpace.PSUM) as pav,
            ):
                def inject_v(rb_i):
                    stv = pst.tile([P, 512], f32, tag="st", name=f"stv{rb_i}")
                    for t in range(CT):
                        mm(stv[:, 0 : NH * D],
                           xts[t][:, rb_i * P : (rb_i + 1) * P],
                           wqs[t][:, 2 * NH * D : 3 * NH * D],
                           start=(t == 0), stop=(t == CT - 1))
                    nc.vector.tensor_copy(
                        vaug[:, rb_i, :, 0:D],
                        stv[:, 0 : NH * D].rearrange("p (h d) -> p h d", h=NH),
                    )

                def inject_qk(g, qtr):
                    # one [128, 512] quarter of a q/k projection group,
                    # accumulated in an st-ring slot then copied to qkT.
                    # Quarter-size keeps the slot hold ~1.3us so the exp
                    # pipeline's double buffering barely degrades.
                    stq = pst.tile([P, 512], f32, tag="st", name=f"stq{g}_{qtr}")
                    xs = slice(qtr * 512, (qtr + 1) * 512)
                    for t in range(CT):
                        mm(stq[:], wqs[t][:, g * P : (g + 1) * P],
                           xts[t][:, xs], start=(t == 0), stop=(t == CT - 1))
                    nc.vector.tensor_copy(qkT[:, g, xs], stq[:])

                def inject_po01(m, half):
                    # phase-3 partial: accumulate pairs 0,1 (g3 = 0,1) of
                    # output-projection m-tile into an st-ring slot, stage
                    # the result in SBUF.  Only the g3 == 2 matmuls and an
                    # add remain for the tail.
                    pq = pst.tile([P, HF], f32, tag="st", name=f"pq{m}_{half}")
                    for g3 in range(2):
                        for qc in range(2):
                            cs = slice(qc * 512, (qc + 1) * 512)
                            acs = slice(half * HF + qc * 512,
                                        half * HF + (qc + 1) * 512)
                            mm(pq[:, cs], wp[:, g3, m * P : (m + 1) * P],
                               attnT[:, g3, acs],
                               start=(g3 == 0), stop=(g3 == 1))
                    nc.vector.tensor_copy(
                        o01[:, m, half * HF : (half + 1) * HF], pq[:]
                    )

                def inject_po2h0(m):
                    # pair-2's contribution to m-tile, q-half 0 (its attnT
                    # is ready once block 4's normalize lands); completes
                    # the half-0 output, which ships to DRAM immediately.
                    pq2 = pst.tile([P, HF], f32, tag="st", name=f"pq2_{m}")
                    for qc in range(2):
                        cs = slice(qc * 512, (qc + 1) * 512)
                        mm(pq2[:, cs], wp[:, 2, m * P : (m + 1) * P],
                           attnT[:, 2, cs], start=True, stop=True)
                    nc.vector.tensor_add(o01[:, m, 0:HF],
                                         pq2[:], o01[:, m, 0:HF])
                    nc.sync.dma_start(outT[m * P : (m + 1) * P, 0:HF],
                                      o01[:, m, 0:HF])

                injections = {}
                for i in range(4, 16):  # v4..v15 just ahead of their AVs
                    injections[(0, i - 3)] = lambda i=i: inject_v(i)
                qk_sched = [
                    (0, 13, 1, 0), (0, 14, 1, 1), (0, 15, 1, 2), (1, 1, 1, 3),
                    (1, 3, 4, 0), (1, 5, 4, 1), (1, 7, 4, 2), (1, 9, 4, 3),
                    (2, 3, 2, 0), (2, 5, 2, 1), (2, 7, 2, 2), (2, 9, 2, 3),
                    (3, 3, 5, 0), (3, 5, 5, 1), (3, 7, 5, 2), (3, 9, 5, 3),
                ]
                for bi_, kb_, g_, q_ in qk_sched:
                    injections[(bi_, kb_)] = lambda g=g_, q=q_: inject_qk(g, q)
                # phase-3 partials over ready pairs keep the late blocks'
                # PE dense once the qk injections are exhausted
                po01_sched = [
                    (3, 13, 0, 0), (3, 15, 1, 0),
                    (4, 3, 2, 0), (4, 5, 3, 0), (4, 7, 4, 0), (4, 9, 5, 0),
                    (4, 11, 0, 1), (4, 13, 1, 1), (4, 15, 2, 1),
                    (5, 1, 3, 1), (5, 3, 4, 1), (5, 5, 5, 1),
                ]
                for bi_, kb_, m_, h_ in po01_sched:
                    injections[(bi_, kb_)] = (
                        lambda m=m_, h=h_: inject_po01(m, h)
                    )
                for i, kb_ in enumerate((7, 9, 11, 13, 15)):
                    injections[(5, kb_)] = lambda m=i: inject_po2h0(m)
                blocks = [(p, hf) for p in range(NP) for hf in range(2)]
                avs = [None] * len(blocks)
                prev = None      # (bi, kb, ptA, ptB)
                pending = []     # deferred normalize tails

                def emit_av(bi, kb, ptA, ptB):
                    p, hf = blocks[bi]
                    avA, avB = avs[bi]
                    for qc in range(2):
                        cs = slice(qc * 512, (qc + 1) * 512)
                        mm(avA[:, cs], vaug[:, kb, 2 * p, :], ptA[:, cs],
                           start=(kb == 0), stop=(kb == KB - 1))
                        mm(avB[:, cs], vaug[:, kb, 2 * p + 1, :], ptB[:, cs],
                           start=(kb == 0), stop=(kb == KB - 1))
                    if kb == KB - 1:
                        emit_norm_a(bi)

                def emit_norm_a(bi):
                    # copy av out of PSUM (frees the accumulator slot), ship
                    # the denominator row to DRAM; the rest is deferred.
                    p, hf = blocks[bi]
                    q0 = hf * HF
                    avA, avB = avs[bi]
                    for off, avX, h in ((0, avA, 2 * p), (D, avB, 2 * p + 1)):
                        un = unp.tile([D + 1, HF], f32, tag="un",
                                      name=f"un{bi}_{h}")
                        nc.vector.tensor_copy(un[:], avX[:])
                        ri = 2 * h + hf
                        nc.sync.dma_start(rdram_s[ri, :], un[D : D + 1, :])
                        pending.append(
                            lambda un=un, ri=ri, off=off, p=p, q0=q0:
                            emit_norm_b(un, ri, off, p, q0)
                        )

                def emit_norm_b(un, ri, off, p, q0):
                    rp = rbp.tile([8, P], f32, tag="rp", name=f"rp{ri}")
                    nc.sync.dma_start(
                        rp[0:8, :],
                        rdram_s[ri, :].rearrange("(a b) -> a b", a=8),
                    )
                    nc.vector.reciprocal(rp[0:8, :], rp[0:8, :])
                    nc.sync.dma_start(rdram_r[ri, :], rp[0:8, :])
                    rb = rbp.tile([D, HF], f32, tag="rb", name=f"rb{ri}")
                    nc.sync.dma_start(
                        rb[:],
                        rdram_r[ri : ri + 1, :].broadcast_to([D, HF]),
                    )
                    nc.vector.tensor_mul(
                        attnT[off : off + D, p, q0 : q0 + HF],
                        un[0:D, :],
                        rb[:],
                    )

                for bi, (p, hf) in enumerate(blocks):
                    q0 = hf * HF
                    hA, hB = 2 * p, 2 * p + 1
                    avs[bi] = (
                        pav.tile([D + 1, HF], f32, tag="av", name=f"avA{bi}"),
                        pav.tile([D + 1, HF], f32, tag="av", name=f"avB{bi}"),
                    )
                    for kb in range(KB):
                        if kb in (4, 5) and pending:
                            fn = pending.pop(0)
                            fn()
                        if (bi, kb) in injections:
                            injections[(bi, kb)]()
                        stA = pst.tile([P, HF], f32, tag="st", name=f"stA{bi}_{kb}")
                        stB = pst.tile([P, HF], f32, tag="st", name=f"stB{bi}_{kb}")
                        for qc in range(2):
                            cs = slice(qc * 512, (qc + 1) * 512)
                            qs = slice(q0 + qc * 512, q0 + (qc + 1) * 512)
                            ks = slice(kb * P, (kb + 1) * P)
                            mm(stA[:, cs], qkT[0:D, NP + p, ks],
                               qkT[0:D, p, qs], start=True, stop=True)
                            mm(stB[:, cs], qkT[D:P, NP + p, ks],
                               qkT[D:P, p, qs], start=True, stop=True)
                        if kb * P // HF == hf:
                            dcol = kb * P - q0
                            dsl = slice(dcol, dcol + P)
                            nc.vector.tensor_mul(
                                stA[:, dsl], stA[:, dsl], masks[:, hA, :]
                            )
                            nc.vector.tensor_mul(
                                stB[:, dsl], stB[:, dsl], masks[:, hB, :]
                            )
                        ptA = ptp.tile([P, HF], bf16, tag="pt", name=f"ptA{bi}_{kb}")
                        nc.scalar.activation(ptA[:], stA[:], Exp, scale=SCALE)
                        ptB = ptp.tile([P, HF], bf16, tag="pt", name=f"ptB{bi}_{kb}")
                        nc.scalar.activation(ptB[:], stB[:], Exp, scale=SCALE)
                        if prev is not None:
                            emit_av(*prev)
                        prev = (bi, kb, ptA, ptB)
                # drain: final AV flush + normalize (its small DMAs go
                # first so they aren't queued behind big output DMAs),
                # last half-0 m-tile, then the half-1 tail.
                emit_av(*prev)
                for fn in pending:
                    fn()
                pending = []
                inject_po2h0(5)
                for m in range(CT):
                    po = pst.tile([P, HF], f32, tag="st", name=f"po2h1_{m}")
                    for qc in range(2):
                        cs = slice(qc * 512, (qc + 1) * 512)
                        acs = slice(HF + qc * 512, HF + (qc + 1) * 512)
                        mm(po[:, cs], wp[:, 2, m * P : (m + 1) * P],
                           attnT[:, 2, acs], start=True, stop=True)
                    nc.vector.tensor_add(o01[:, m, HF:N],
                                         po[:], o01[:, m, HF:N])
                    nc.sync.dma_start(outT[m * P : (m + 1) * P, HF:N],
                                      o01[:, m, HF:N])
            qin_cm.__exit__(None, None, None)

    if not nc.is_finalized():
        nc.finalize()
    return nc


def _get_program():
    if "nc" not in _CACHE:
        _CACHE["nc"] = _build_program()
    return _CACHE["nc"]


def _in_maps(x, w_qkv, w_proj, temperature):
    import ml_dtypes

    bf16 = ml_dtypes.bfloat16
    t = np.asarray(temperature, dtype=np.float32).reshape(H)
    maps = []
    xTs = {}
    for c in range(8):
        b, h0 = c // 2, NH * (c % 2)
        if b not in xTs:
            xTs[b] = np.ascontiguousarray(
                np.asarray(x[b], dtype=np.float32).T.astype(bf16)
            )
        cols = slice(D * h0, D * h0 + NH * D)
        wq = np.concatenate(
            [w_qkv[:, cols], w_qkv[:, C:][:, cols], w_qkv[:, 2 * C :][:, cols]],
            axis=1,
        )
        maps.append(
            {
                "xT": xTs[b],
                "wqkv": np.ascontiguousarray(wq).astype(bf16),
                "wproj": np.ascontiguousarray(
                    w_proj[D * h0 : D * h0 + NH * D, :]
                ).astype(bf16),
                "temp": np.ascontiguousarray(
                    np.broadcast_to(t[h0 : h0 + NH].reshape(1, NH), (P, NH))
                ),
            }
        )
    return maps


def _install_profile_hook():
    """The agent image's antenv lacks axon_hooks; synthesize it and register
    the ctypes NTFF hook so run_bass_kernel_spmd(trace=True) can profile."""
    import types, importlib

    if "antenv.axon_hooks" not in sys.modules:
        import antenv

        mod = types.ModuleType("antenv.axon_hooks")
        _state = {"hook": None}
        mod.set_axon_ntff_profile_hook = lambda h: _state.__setitem__("hook", h)
        mod.get_axon_ntff_profile_hook = lambda: _state["hook"]
        sys.modules["antenv.axon_hooks"] = mod
        antenv.axon_hooks = mod
    from antenv.axon_hooks import (
        get_axon_ntff_profile_hook,
        set_axon_ntff_profile_hook,
    )

    if get_axon_ntff_profile_hook() is None:
        tb = importlib.import_module("trn_agent_boot.trn_boot")
        hook = tb._ntff_profile_via_ctypes("/opt/axon/libaxon_pjrt.so")
        set_axon_ntff_profile_hook(hook)


def kernel(x, w_qkv, w_proj, b_proj, temperature, _trace=False):
    from concourse.bass_utils import run_bass_kernel_spmd

    if _trace:
        try:
            _install_profile_hook()
        except Exception as e:  # profiling is best-effort
            print(f"profile hook install failed: {e}")

    nc = _get_program()
    maps = _in_maps(
        np.asarray(x, np.float32),
        np.asarray(w_qkv, np.float32),
        np.asarray(w_proj, np.float32),
        np.asarray(temperature, np.float32),
    )
    res = run_bass_kernel_spmd(nc, maps, list(range(8)), trace=_trace)
    parts = [r["outT"] for r in res.results]
    bp = np.asarray(b_proj, np.float32)
    out = np.stack(
        [(parts[2 * b] + parts[2 * b + 1]).T + bp for b in range(B)]
    ).astype(np.float32)
    if _trace:
        _CACHE["last_result"] = res
    return out


# revision 47
# speedup vs baseline: 1.4901x; 1.0230x over previous
"""LocalitySelfAttention TRN2 kernel (v6: flat cross-block pipeline).

B=4, N=2048, C=768, H=12, D=64.  8 cores: core c -> batch c//2, heads
6*(c%2) .. 6*(c%2)+6 (6 contiguous heads = 3 pairs).  Each core computes its
heads' qkv projection, attention, and a partial output projection restricted
to its heads' 384 rows of w_proj.  Host sums the two partials per batch and
adds b_proj.

The whole phase-2 is ONE flat software pipeline over (pair, q-half, kb)
iterations with the AV matmuls lagging the score matmuls by exactly one
iteration, including across block boundaries.  This keeps the PE's strict
in-order queue free of semaphore waits (an AV never reaches the queue head
before its exp finished) and keeps PE activity dense enough that the HAM
clock gate stays at full rate.  Softmax normalization runs entirely off the
critical path: denominator rows round-trip through DRAM (reshaped [8,128]
so the DVE reciprocal runs on 8 partitions at free=128), with the dependent
DVE ops deferred half an iteration-block so their DMA waits are always
pre-satisfied when they reach the strict-FIFO DVE queue.
"""

import sys
import numpy as np

if "/opt/trn_rl_repo" not in sys.path:
    sys.path.insert(0, "/opt/trn_rl_repo")

B, N, C, H = 4, 2048, 768, 12
D = C // H          # 64
NH = 6              # heads per core
NP = NH // 2        # head pairs per core = 3
P = 128
CT = C // P         # 6 contraction tiles
KB = N // P         # 16 key blocks
QC = N // 512       # 4 free-dim chunks of 512
HF = 1024           # q processed in halves
SCALE = float(D) ** -0.5  # 0.125

_CACHE = {}


def _build_program():
    import concourse.bass as bass
    import concourse.mybir as mybir
    import concourse.tile as tile
    from concourse import bacc
    from concourse.masks import make_identity

    f32 = mybir.dt.float32
    bf16 = mybir.dt.bfloat16
    Exp = mybir.ActivationFunctionType.Exp
    mult = mybir.AluOpType.mult
    add = mybir.AluOpType.add

    nc = bacc.Bacc()
    xT = nc.dram_tensor("xT", [C, N], bf16, kind="ExternalInput")
    wqkv = nc.dram_tensor("wqkv", [C, 3 * NH * D], bf16, kind="ExternalInput")
    wproj = nc.dram_tensor("wproj", [NH * D, C], bf16, kind="ExternalInput")
    temp = nc.dram_tensor("temp", [P, NH], f32, kind="ExternalInput")
    outT = nc.dram_tensor("outT", [C, N], f32, kind="ExternalOutput")
    rdram_s = nc.dram_tensor("rscratch_s", [2 * NH, HF], f32)  # denom rows
    rdram_r = nc.dram_tensor("rscratch_r", [2 * NH, HF], f32)  # recip rows

    mm = nc.tensor.matmul

    with tile.TileContext(nc) as tc:
        with (
            tc.tile_pool(name="const", bufs=1) as constp,
            tc.tile_pool(name="persist", bufs=1) as persist,
        ):
            # ---- setup: temperature diag masks (1 - t_h * I) ----------
            ident = constp.tile([P, P], f32, tag="ident")
            make_identity(nc, ident[:])
            tbc = constp.tile([P, NH], f32, tag="tbc")
            nc.sync.dma_start(tbc[:, :], temp[:, :])
            ntb = constp.tile([P, NH], f32, tag="ntb")
            nc.vector.tensor_scalar_mul(ntb[:, :], tbc[:, :], -1.0)
            masks = constp.tile([P, NH, P], f32, tag="masks")
            for h in range(NH):
                nc.vector.tensor_scalar(
                    masks[:, h, :], ident[:], ntb[:, h : h + 1], 1.0, mult, add
                )

            # persistent: qT/kT (head pairs stacked on partitions), v_aug
            qkT = persist.tile([P, 2 * NP, N], bf16, tag="qkT")  # 0-2 q, 3-5 k
            vaug = persist.tile([P, KB, NH, D + 1], bf16, tag="vaug")
            onesrc = constp.tile([P, KB * NH], f32, tag="onesrc")
            nc.vector.memset(onesrc[:], 1.0)
            nc.vector.tensor_copy(
                vaug[:, :, :, D : D + 1],
                onesrc[:].rearrange("p (a b c) -> p a b c", a=KB, b=NH),
            )
            wp = persist.tile([P, NH * D // P, C], bf16, tag="wp")  # [128,3,768]
            for g3 in range(NH * D // P):
                nc.scalar.dma_start(wp[:, g3, :], wproj[g3 * P : (g3 + 1) * P, :])
            attnT = persist.tile([P, NP, N], bf16, tag="attnT")
            o01 = persist.tile([P, CT, N], f32, tag="o01")  # phase-3 partials

            # ---- phase 1 (prefix): v + pair-0's q/k only --------------
            # The remaining q/k projection groups are injected into the
            # phase-2 pipeline at intervals: each injection is a dense,
            # exp-independent PE burst that re-warms the HAM clock gate
            # (an ACT-bound steady state alone never has a 3.4us
            # contiguous-busy window, so the PE would stay cold forever).
            qin_cm = tc.tile_pool(name="qin", bufs=1)
            qin = qin_cm.__enter__()
            xts, wqs = [], []
            for t in range(CT):
                xti = qin.tile([P, N], bf16, tag=f"xt{t}", name=f"xt{t}")
                nc.sync.dma_start(xti[:], xT[t * P : (t + 1) * P, :])
                xts.append(xti)
                wqi = qin.tile([P, 3 * NH * D], bf16, tag=f"wq{t}", name=f"wq{t}")
                nc.sync.dma_start(wqi[:], wqkv[t * P : (t + 1) * P, :])
                wqs.append(wqi)

            with tc.tile_pool(name="psum1", bufs=2, space=bass.MemorySpace.PSUM) as psum1:

                def qk_group(g):
                    ps = psum1.tile([P, N], f32, tag="ps", name=f"ps{g}")
                    for t in range(CT):
                        for qc in range(QC):
                            mm(
                                ps[:, qc * 512 : (qc + 1) * 512],
                                wqs[t][:, g * P : (g + 1) * P],
                                xts[t][:, qc * 512 : (qc + 1) * 512],
                                start=(t == 0),
                                stop=(t == CT - 1),
                            )
                    nc.vector.tensor_copy(qkT[:, g, :], ps[:])

                def v_group(rb_i):
                    psv = psum1.tile([P, NH * D], f32, tag="ps", name=f"psv{rb_i}")
                    for t in range(CT):
                        mm(
                            psv[:],
                            xts[t][:, rb_i * P : (rb_i + 1) * P],
                            wqs[t][:, 2 * NH * D : 3 * NH * D],
                            start=(t == 0),
                            stop=(t == CT - 1),
                        )
                    nc.vector.tensor_copy(
                        vaug[:, rb_i, :, 0:D],
                        psv[:].rearrange("p (h d) -> p h d", h=NH),
                    )

                # q/k of pair 0 and the first few v groups only: the PE
                # queue is strictly in-order, so anything emitted here
                # delays the first exp.  v4-v15 are injected into early
                # block-0 iterations, just ahead of their AV consumers.
                qk_group(0)
                qk_group(3)
                for i in range(4):
                    v_group(i)

            # ---- phase 2: attention, one flat pipeline ----------------
            with (
                tc.tile_pool(name="pt", bufs=4) as ptp,
                tc.tile_pool(name="un", bufs=4) as unp,
                tc.tile_pool(name="rb", bufs=2) as rbp,
                tc.tile_pool(name="pst", bufs=2, space=bass.MemorySpace.PSUM) as pst,
                tc.tile_pool(name="pav", bufs=2, space=bass.MemorySpace.PSUM) as pav,
            ):
                def inject_v(rb_i):
                    stv = pst.tile([P, 512], f32, tag="st", name=f"stv{rb_i}")
                    for t in range(CT):
                        mm(stv[:, 0 : NH * D],
                           xts[t][:, rb_i * P : (rb_i + 1) * P],
                           wqs[t][:, 2 * NH * D : 3 * NH * D],
                           start=(t == 0), stop=(t == CT - 1))
                    nc.vector.tensor_copy(
                        vaug[:, rb_i, :, 0:D],
                        stv[:, 0 : NH * D].rearrange("p (h d) -> p h d", h=NH),
                    )

                def inject_qk(g, qtr):
                    # one [128, 512] quarter of a q/k projection group,
                    # accumulated in an st-ring slot then copied to qkT.
                    # Quarter-size keeps the slot hold ~1.3us so the exp
                    # pipeline's double buffering barely degrades.
                    stq = pst.tile([P, 512], f32, tag="st", name=f"stq{g}_{qtr}")
                    xs = slice(qtr * 512, (qtr + 1) * 512)
                    for t in range(CT):
                        mm(stq[:], wqs[t][:, g * P : (g + 1) * P],
                           xts[t][:, xs], start=(t == 0), stop=(t == CT - 1))
                    nc.vector.tensor_copy(qkT[:, g, xs], stq[:])

                def inject_po01(m, half):
                    # phase-3 partial: accumulate pairs 0,1 (g3 = 0,1) of
                    # output-projection m-tile into an st-ring slot, stage
                    # the result in SBUF.  Only the g3 == 2 matmuls and an
                    # add remain for the tail.
                    pq = pst.tile([P, HF], f32, tag="st", name=f"pq{m}_{half}")
                    for g3 in range(2):
                        for qc in range(2):
                            cs = slice(qc * 512, (qc + 1) * 512)
                            acs = slice(half * HF + qc * 512,
                                        half * HF + (qc + 1) * 512)
                            mm(pq[:, cs], wp[:, g3, m * P : (m + 1) * P],
                               attnT[:, g3, acs],
                               start=(g3 == 0), stop=(g3 == 1))
                    nc.vector.tensor_copy(
                        o01[:, m, half * HF : (half + 1) * HF], pq[:]
                    )

                def inject_po2h0(m):
                    # pair-2's contribution to m-tile, q-half 0 (its attnT
                    # is ready once block 4's normalize lands); completes
                    # the half-0 output, which ships to DRAM immediately.
                    pq2 = pst.tile([P, HF], f32, tag="st", name=f"pq2_{m}")
                    for qc in range(2):
                        cs = slice(qc * 512, (qc + 1) * 512)
                        mm(pq2[:, cs], wp[:, 2, m * P : (m + 1) * P],
                           attnT[:, 2, cs], start=True, stop=True)
                    nc.vector.tensor_add(o01[:, m, 0:HF],
                                         pq2[:], o01[:, m, 0:HF])
                    nc.sync.dma_start(outT[m * P : (m + 1) * P, 0:HF],
                                      o01[:, m, 0:HF])

                injections = {}
                for i in range(4, 16):  # v4..v15 just ahead of their AVs
                    injections[(0, i - 3)] = lambda i=i: inject_v(i)
                qk_sched = [
                    (0, 13, 1, 0), (0, 14, 1, 1), (0, 15, 1, 2), (1, 1, 1, 3),
                    (1, 3, 4, 0), (1, 5, 4, 1), (1, 7, 4, 2), (1, 9, 4, 3),
                    (2, 3, 2, 0), (2, 5, 2, 1), (2, 7, 2, 2), (2, 9, 2, 3),
                    (3, 3, 5, 0), (3, 5, 5, 1), (3, 7, 5, 2), (3, 9, 5, 3),
                ]
                for bi_, kb_, g_, q_ in qk_sched:
                    injections[(bi_, kb_)] = lambda g=g_, q=q_: inject_qk(g, q)
                # phase-3 partials over ready pairs keep the late blocks'
                # PE dense once the qk injections are exhausted
                po01_sched = [
                    (3, 13, 0, 0), (3, 15, 1, 0),
                    (4, 3, 2, 0), (4, 5, 3, 0), (4, 7, 4, 0), (4, 9, 5, 0),
                    (4, 11, 0, 1), (4, 13, 1, 1), (4, 15, 2, 1),
                    (5, 1, 3, 1), (5, 3, 4, 1), (5, 5, 5, 1),
                ]
                for bi_, kb_, m_, h_ in po01_sched:
                    injections[(bi_, kb_)] = (
                        lambda m=m_, h=h_: inject_po01(m, h)
                    )
                for i, kb_ in enumerate((7, 9, 11, 13, 15)):
                    injections[(5, kb_)] = lambda m=i: inject_po2h0(m)
                blocks = [(p, hf) for p in range(NP) for hf in range(2)]
                avs = [None] * len(blocks)
                prev = None      # (bi, kb, ptA, ptB)
                pending = []     # deferred normalize tails

                def emit_av(bi, kb, ptA, ptB):
                    p, hf = blocks[bi]
                    avA, avB = avs[bi]
                    for qc in range(2):
                        cs = slice(qc * 512, (qc + 1) * 512)
                        mm(avA[:, cs], vaug[:, kb, 2 * p, :], ptA[:, cs],
                           start=(kb == 0), stop=(kb == KB - 1))
                        mm(avB[:, cs], vaug[:, kb, 2 * p + 1, :], ptB[:, cs],
                           start=(kb == 0), stop=(kb == KB - 1))
                    if kb == KB - 1:
                        emit_norm_a(bi)

                def emit_norm_a(bi):
                    # copy av out of PSUM (frees the accumulator slot), ship
                    # the denominator row to DRAM; the rest is deferred.
                    p, hf = blocks[bi]
                    q0 = hf * HF
                    avA, avB = avs[bi]
                    for off, avX, h in ((0, avA, 2 * p), (D, avB, 2 * p + 1)):
                        un = unp.tile([D + 1, HF], f32, tag="un",
                                      name=f"un{bi}_{h}")
                        nc.vector.tensor_copy(un[:], avX[:])
                        ri = 2 * h + hf
                        nc.sync.dma_start(rdram_s[ri, :], un[D : D + 1, :])
                        pending.append(
                            lambda fast=False, un=un, ri=ri, off=off, p=p, q0=q0:
                            emit_norm_b(un, ri, off, p, q0, fast)
                        )

                def emit_norm_b(un, ri, off, p, q0, fast=False):
                    # fast=True (kernel drain): the DMA chain rides the
                    # Activation engine's queue, which is idle by then and
                    # not backed up behind the big output DMAs.
                    dma = nc.scalar.dma_start if fast else nc.sync.dma_start
                    rp = rbp.tile([8, P], f32, tag="rp", name=f"rp{ri}")
                    dma(
                        rp[0:8, :],
                        rdram_s[ri, :].rearrange("(a b) -> a b", a=8),
                    )
                    nc.vector.reciprocal(rp[0:8, :], rp[0:8, :])
                    dma(rdram_r[ri, :], rp[0:8, :])
                    rb = rbp.tile([D, HF], f32, tag="rb", name=f"rb{ri}")
                    dma(
                        rb[:],
                        rdram_r[ri : ri + 1, :].broadcast_to([D, HF]),
                    )
                    nc.vector.tensor_mul(
                        attnT[off : off + D, p, q0 : q0 + HF],
                        un[0:D, :],
                        rb[:],
                    )

                for bi, (p, hf) in enumerate(blocks):
                    q0 = hf * HF
                    hA, hB = 2 * p, 2 * p + 1
                    avs[bi] = (
                        pav.tile([D + 1, HF], f32, tag="av", name=f"avA{bi}"),
                        pav.tile([D + 1, HF], f32, tag="av", name=f"avB{bi}"),
                    )
                    for kb in range(KB):
                        if kb in (4, 5) and pending:
                            fn = pending.pop(0)
                            fn()
                        if (bi, kb) in injections:
                            injections[(bi, kb)]()
                        stA = pst.tile([P, HF], f32, tag="st", name=f"stA{bi}_{kb}")
                        stB = pst.tile([P, HF], f32, tag="st", name=f"stB{bi}_{kb}")
                        for qc in range(2):
                            cs = slice(qc * 512, (qc + 1) * 512)
                            qs = slice(q0 + qc * 512, q0 + (qc + 1) * 512)
                            ks = slice(kb * P, (kb + 1) * P)
                            mm(stA[:, cs], qkT[0:D, NP + p, ks],
                               qkT[0:D, p, qs], start=True, stop=True)
                            mm(stB[:, cs], qkT[D:P, NP + p, ks],
                               qkT[D:P, p, qs], start=True, stop=True)
                        if kb * P // HF == hf:
                            dcol = kb * P - q0
                            dsl = slice(dcol, dcol + P)
                            nc.vector.tensor_mul(
                                stA[:, dsl], stA[:, dsl], masks[:, hA, :]
                            )
                            nc.vector.tensor_mul(
                                stB[:, dsl], stB[:, dsl], masks[:, hB, :]
                            )
                        ptA = ptp.tile([P, HF], bf16, tag="pt", name=f"ptA{bi}_{kb}")
                        nc.scalar.activation(ptA[:], stA[:], Exp, scale=SCALE)
                        ptB = ptp.tile([P, HF], bf16, tag="pt", name=f"ptB{bi}_{kb}")
                        nc.scalar.activation(ptB[:], stB[:], Exp, scale=SCALE)
                        if prev is not None:
                            emit_av(*prev)
                        prev = (bi, kb, ptA, ptB)
                # drain: final AV flush + normalize (its small DMAs go
                # first so they aren't queued behind big output DMAs),
                # last half-0 m-tile, then the half-1 tail.
                emit_av(*prev)
                for fn in pending:
                    fn(True)
                pending = []
                inject_po2h0(5)
                for m in range(CT):
                    po = pst.tile([P, HF], f32, tag="st", name=f"po2h1_{m}")
                    for qc in range(2):
                        cs = slice(qc * 512, (qc + 1) * 512)
                        acs = slice(HF + qc * 512, HF + (qc + 1) * 512)
                        mm(po[:, cs], wp[:, 2, m * P : (m + 1) * P],
                           attnT[:, 2, acs], start=True, stop=True)
                    nc.vector.tensor_add(o01[:, m, HF:N],
                                         po[:], o01[:, m, HF:N])
                    dma = nc.sync.dma_start if m % 2 else nc.scalar.dma_start
                    dma(outT[m * P : (m + 1) * P, HF:N], o01[:, m, HF:N])
            qin_cm.__exit__(None, None, None)

    if not nc.is_finalized():
        nc.finalize()
    return nc


def _get_program():
    if "nc" not in _CACHE:
        _CACHE["nc"] = _build_program()
    return _CACHE["nc"]


def _in_maps(x, w_qkv, w_proj, temperature):
    import ml_dtypes

    bf16 = ml_dtypes.bfloat16
    t = np.asarray(temperature, dtype=np.float32).reshape(H)
    maps = []
    xTs = {}
    for c in range(8):
        b, h0 = c // 2, NH * (c % 2)
        if b not in xTs:
            xTs[b] = np.ascontiguousarray(
                np.asarray(x[b], dtype=np.float32).T.astype(bf16)
            )
        cols = slice(D * h0, D * h0 + NH * D)
        wq = np.concatenate(
            [w_qkv[:, cols], w_qkv[:, C:][:, cols], w_qkv[:, 2 * C :][:, cols]],
            axis=1,
        )
        maps.append(
            {
                "xT": xTs[b],
                "wqkv": np.ascontiguousarray(wq).astype(bf16),
                "wproj": np.ascontiguousarray(
                    w_proj[D * h0 : D * h0 + NH * D, :]
                ).astype(bf16),
                "temp": np.ascontiguousarray(
                    np.broadcast_to(t[h0 : h0 + NH].reshape(1, NH), (P, NH))
                ),
            }
        )
    return maps


def _install_profile_hook():
    """The agent image's antenv lacks axon_hooks; synthesize it and register
    the ctypes NTFF hook so run_bass_kernel_spmd(trace=True) can profile."""
    import types, importlib

    if "antenv.axon_hooks" not in sys.modules:
        import antenv

        mod = types.ModuleType("antenv.axon_hooks")
        _state = {"hook": None}
        mod.set_axon_ntff_profile_hook = lambda h: _state.__setitem__("hook", h)
        mod.get_axon_ntff_profile_hook = lambda: _state["hook"]
        sys.modules["antenv.axon_hooks"] = mod
        antenv.axon_hooks = mod
    from antenv.axon_hooks import (
        get_axon_ntff_profile_hook,
        set_axon_ntff_profile_hook,
    )

    if get_axon_ntff_profile_hook() is None:
        tb = importlib.import_module("trn_agent_boot.trn_boot")
        hook = tb._ntff_profile_via_ctypes("/opt/axon/libaxon_pjrt.so")
        set_axon_ntff_profile_hook(hook)


def kernel(x, w_qkv, w_proj, b_proj, temperature, _trace=False):
    from concourse.bass_utils import run_bass_kernel_spmd

    if _trace:
        try:
            _install_profile_hook()
        except Exception as e:  # profiling is best-effort
            print(f"profile hook install failed: {e}")

    nc = _get_program()
    maps = _in_maps(
        np.asarray(x, np.float32),
        np.asarray(w_qkv, np.float32),
        np.asarray(w_proj, np.float32),
        np.asarray(temperature, np.float32),
    )
    res = run_bass_kernel_spmd(nc, maps, list(range(8)), trace=_trace)
    parts = [r["outT"] for r in res.results]
    bp = np.asarray(b_proj, np.float32)
    out = np.stack(
        [(parts[2 * b] + parts[2 * b + 1]).T + bp for b in range(B)]
    ).astype(np.float32)
    if _trace:
        _CACHE["last_result"] = res
    return out
